# revision 16
# baseline (speedup 1.0000x reference)
"""DocRE model kernel for 8 Trainium2 NeuronCores.

Data-parallel over the pair grid: core = b*4 + ib owns document b and
i-rows [8*ib, 8*ib+8) of the 32x32 entity-pair grid (256 pairs/core).
All weights are replicated; W_ext (49152x768) is streamed from HBM.

Phase M uses "orientation B" matmuls: the stationary operand is a
128x128 chunk of W_ext and the moving operand is the group-bilinear
feature tile bl [k=128, 256 pairs], accumulating feat^T
[emb-dim, pairs] across all 384 k-tiles.  The bl tile is built on the
DVE as hsdup * tsrep where the j-side broadcast (2 -> 128 partitions)
runs on the otherwise-idle GPSIMD engine (partition_broadcast), not
the PE.  W_ext rows are host-permuted to (g, t, jh, i) so each k-tile
holds j in {2t, 2t+1} interleaved across partition halves.
"""

import numpy as np

import concourse.bacc as bacc
import concourse.bass as bass
import concourse.tile as tile
from concourse import mybir
from concourse.bass_utils import run_bass_kernel_spmd
from concourse.masks import make_identity

F32 = mybir.dt.float32
F16 = mybir.dt.float16

B, L, H = 2, 1024, 768
E, M = 32, 4
EMB, BLK, NL = 768, 64, 97
G = EMB // BLK  # 12
LN_EPS = 1e-12

N_CORES = 8
IB = E // (N_CORES // B)     # 8 i-rows per core
NPAIR = IB * E               # 256 pairs per core
KT = EMB * BLK // 128        # 384 k-tiles
TPG = KT // G                # 32 k-tiles per group
CT = EMB // 128              # 6 feature chunks
KC = H // 128                # 6 contraction chunks of H
LC = L // 128                # 8 chunks of L
NENT = IB + E + 1            # 41 cols: [my 8 entities | all 32 | cls]
NE2 = NENT + 1


def _build_module():
    nc = bacc.Bacc("TRN2", target_bir_lowering=False, debug=False)

    seq_d = nc.dram_tensor("seq", [L, H], F16, kind="ExternalInput")
    S_d = nc.dram_tensor("S", [L, NENT], F16, kind="ExternalInput")
    Wh_d = nc.dram_tensor("Wh", [3 * H, EMB], F16, kind="ExternalInput")
    Wt_d = nc.dram_tensor("Wt", [3 * H, EMB], F16, kind="ExternalInput")
    bh_d = nc.dram_tensor("bh", [128, CT], F32, kind="ExternalInput")
    bt_d = nc.dram_tensor("bt", [128, CT], F32, kind="ExternalInput")
    Wx_d = nc.dram_tensor("Wx", [EMB * BLK, EMB], F16, kind="ExternalInput")
    bx_d = nc.dram_tensor("bx", [128, CT], F32, kind="ExternalInput")
    lng_d = nc.dram_tensor("lng", [128, CT], F32, kind="ExternalInput")
    lnb_d = nc.dram_tensor("lnb", [128, CT], F32, kind="ExternalInput")
    Wc_d = nc.dram_tensor("Wc", [EMB, NL], F32, kind="ExternalInput")
    out_d = nc.dram_tensor("out", [NPAIR, NL], F32, kind="ExternalOutput")

    with tile.TileContext(nc) as tc:
        with (
            tc.tile_pool(name="persist", bufs=1) as persist,
            tc.tile_pool(name="seqp", bufs=1) as seqp,
            tc.tile_pool(name="whp", bufs=9) as whp,
            tc.tile_pool(name="wxp", bufs=20) as wxp,
            tc.tile_pool(name="blp", bufs=6) as blp,
            tc.tile_pool(name="hsdupp", bufs=2) as hsdupp,
            tc.tile_pool(name="tspp", bufs=2) as tspp,
            tc.tile_pool(name="tsrepp", bufs=2) as tsrepp,
            tc.tile_pool(name="tsop", bufs=2) as tsop,
            tc.tile_pool(name="sqp", bufs=2) as sqp,
            tc.tile_pool(name="smallp", bufs=1) as smallp,
            tc.tile_pool(name="tmpp", bufs=2) as tmpp,
            tc.tile_pool(name="dramp", bufs=1, space="DRAM") as dramp,
            tc.tile_pool(name="psf", bufs=1, space="PSUM") as psf,
            tc.tile_pool(name="psg", bufs=2, space="PSUM") as psg,
        ):
            ident = persist.tile([128, 128], F32, name="ident")
            make_identity(nc, ident[:])

            # ---- per-partition-per-chunk constants [128, CT] ----
            bh_t = persist.tile([128, CT], F32, name="bh_t")
            bt_t = persist.tile([128, CT], F32, name="bt_t")
            bx_t = persist.tile([128, CT], F32, name="bx_t")
            lng_t = persist.tile([128, CT], F32, name="lng_t")
            lnb_t = persist.tile([128, CT], F32, name="lnb_t")
            for tile_, src in ((bh_t, bh_d), (bt_t, bt_d), (bx_t, bx_d),
                               (lng_t, lng_d), (lnb_t, lnb_d)):
                nc.sync.dma_start(tile_[:], src.ap())

            eps_t = persist.tile([128, 1], F32, name="eps")
            nc.vector.memset(eps_t[:], LN_EPS)
            ones_t = persist.tile([128, 1], F32, name="ones")
            nc.vector.memset(ones_t[:], 1.0)

            wc_t = persist.tile([128, CT, NL], F32, name="wc_t")
            nc.sync.dma_start(wc_t[:], Wc_d.ap().rearrange("(c p) n -> p c n", p=128))

            # ---- phase E: entity pooling  ent = S^T @ seq ----
            seq_t = seqp.tile([128, LC, H], F16, name="seq_t")
            S_t = seqp.tile([128, LC, NENT], F16, name="S_t")
            seq_re = seq_d.ap().rearrange("(c p) h -> p c h", p=128)
            S_re = S_d.ap().rearrange("(c p) n -> p c n", p=128)
            for kc in range(LC):
                nc.sync.dma_start(S_t[:, kc, :], S_re[:, kc, :])
                nc.sync.dma_start(seq_t[:, kc, :], seq_re[:, kc, :])

            ps_e0 = psg.tile([NENT, 512], F32, name="gen")
            ps_e1 = psg.tile([NENT, 256], F32, name="gen")
            for kc in range(LC):
                nc.tensor.matmul(ps_e0[:], S_t[:, kc, :], seq_t[:, kc, 0:512],
                                 start=(kc == 0), stop=(kc == LC - 1))
                nc.tensor.matmul(ps_e1[:], S_t[:, kc, :], seq_t[:, kc, 512:768],
                                 start=(kc == 0), stop=(kc == LC - 1))
            ent_nat = persist.tile([NENT, H], F32, name="ent_nat")
            nc.scalar.copy(ent_nat[:, 0:512], ps_e0[:])
            nc.scalar.copy(ent_nat[:, 512:768], ps_e1[:])

            # transpose ent -> entT [h, NENT] (stationary for projections)
            entT = persist.tile([128, KC, NENT], F16, name="entT")
            for kc in range(KC):
                ps_tr = psg.tile([128, NENT], F32, name="gen")
                nc.tensor.transpose(ps_tr[:], ent_nat[:, kc * 128:(kc + 1) * 128],
                                    ident[:NENT, :NENT])
                nc.scalar.copy(entT[:, kc, :], ps_tr[:])

            # ---- phase A: A/B/C projections ----
            ABCD = []
            for ct in range(CT):
                abcd_alloc = persist.tile([128, 4, NE2], F32, name=f"abcd{ct}")
                nc.vector.memset(abcd_alloc[:], 0.0)
                ABCD.append(abcd_alloc)

            def emit_ab_chain(m, w_d, blk):
                ps_n0 = psg.tile([NENT, 512], F32, name="gen")
                ps_n1 = psg.tile([NENT, 256], F32, name="gen")
                for kc in range(KC):
                    w_t = whp.tile([128, EMB], F16, name="w_t")
                    nc.sync.dma_start(
                        w_t[:], w_d.ap()[blk * H + kc * 128: blk * H + (kc + 1) * 128, :])
                    nc.tensor.matmul(ps_n0[:], entT[:, kc, :], w_t[:, 0:512],
                                     start=(kc == 0), stop=(kc == KC - 1))
                    nc.tensor.matmul(ps_n1[:], entT[:, kc, :], w_t[:, 512:768],
                                     start=(kc == 0), stop=(kc == KC - 1))
                x_nat = tmpp.tile([NENT, EMB], F32, name="x_nat")
                nc.scalar.copy(x_nat[:, 0:512], ps_n0[:])
                nc.scalar.copy(x_nat[:, 512:768], ps_n1[:])
                for ct in range(CT):
                    ps_tr = psg.tile([128, NENT], F32, name="gen")
                    nc.tensor.transpose(ps_tr[:], x_nat[:, ct * 128:(ct + 1) * 128],
                                        ident[:NENT, :NENT])
                    nc.scalar.copy(ABCD[ct][:, m, 0:NENT], ps_tr[:])

            def emit_c_chain(m_sel, w_d, bias_t):
                ps_c0 = psg.tile([NENT, 512], F32, name="gen")
                ps_c1 = psg.tile([NENT, 256], F32, name="gen")
                for kc in range(KC):
                    w_t = whp.tile([128, EMB], F16, name="w_t")
                    nc.sync.dma_start(
                        w_t[:], w_d.ap()[2 * H + kc * 128: 2 * H + (kc + 1) * 128, :])
                    nc.tensor.matmul(ps_c0[:1, :], entT[:, kc, IB + E:IB + E + 1],
                                     w_t[:, 0:512],
                                     start=(kc == 0), stop=(kc == KC - 1))
                    nc.tensor.matmul(ps_c1[:1, :], entT[:, kc, IB + E:IB + E + 1],
                                     w_t[:, 512:768],
                                     start=(kc == 0), stop=(kc == KC - 1))
                c_nat = tmpp.tile([1, EMB], F32, name="c_nat")
                nc.scalar.copy(c_nat[:, 0:512], ps_c0[:1, :])
                nc.scalar.copy(c_nat[:, 512:768], ps_c1[:1, :])
                for ct in range(CT):
                    ps_tr = psg.tile([128, NENT], F32, name="gen")
                    nc.tensor.transpose(ps_tr[:, 0:1],
                                        c_nat[:, ct * 128:(ct + 1) * 128],
                                        ident[:1, :1])
                    nc.vector.tensor_tensor(ABCD[ct][:, m_sel, NENT:NENT + 1],
                                            ps_tr[:, 0:1],
                                            bias_t[:, ct:ct + 1],
                                            op=mybir.AluOpType.add)

            def colview(tile_, m, col0, ap_pat):
                return bass.AP(tensor=tile_.tensor,
                               offset=tile_.offset + m * NE2 + col0,
                               ap=[tile_.ap[0]] + ap_pat)

            # ---- tail-side projections -> tsT (natural [emb, pair]) ----
            emit_c_chain(3, Wt_d, bt_t)
            emit_ab_chain(2, Wt_d, 0)
            emit_ab_chain(3, Wt_d, 1)

            tsT = persist.tile([128, CT, 2 * 128], F16, name="tsT")
            ts_dram = dramp.tile([EMB, 2 * 128], F16, name="ts_dram")
            for ct in range(CT):
                abcd_t = ABCD[ct]
                tmp2 = tmpp.tile([128, 8, 32], F32, name="tmp2")
                nc.vector.tensor_tensor(
                    tmp2[:], colview(abcd_t, 2, IB, [[0, 8], [1, 32]]),
                    colview(abcd_t, 3, 0, [[1, 8], [0, 32]]),
                    op=mybir.AluOpType.add)
                nc.scalar.activation(
                    tsT[:, ct, :].rearrange("p (a b) -> p a b", a=8),
                    tmp2[:], mybir.ActivationFunctionType.Tanh,
                    bias=abcd_t[:, 3, NENT:NENT + 1], scale=1.0)
                nc.scalar.dma_start(ts_dram[ct * 128:(ct + 1) * 128, :],
                                    tsT[:, ct, :])

            # ---- head-side projections -> hsT ----
            emit_c_chain(0, Wh_d, bh_t)
            emit_ab_chain(0, Wh_d, 0)
            emit_ab_chain(1, Wh_d, 1)

            hsT = persist.tile([128, CT, 2 * 128], F16, name="hsT")
            for ct in range(CT):
                abcd_t = ABCD[ct]
                tmp = tmpp.tile([128, 8, 32], F32, name="tmp")
                nc.vector.tensor_tensor(
                    tmp[:], colview(abcd_t, 0, 0, [[1, 8], [0, 32]]),
                    colview(abcd_t, 1, IB, [[0, 8], [1, 32]]),
                    op=mybir.AluOpType.add)
                nc.scalar.activation(
                    hsT[:, ct, :].rearrange("p (a b) -> p a b", a=8),
                    tmp[:], mybir.ActivationFunctionType.Tanh,
                    bias=abcd_t[:, 0, NENT:NENT + 1], scale=1.0)

            # ---- phase M: main contraction over W_ext ----
            # feat^T accumulators: 6 chunks of [128 emb-dims, 256 pairs]
            ps_feat = [psf.tile([128, 256], F32, name=f"pf{oc}")
                       for oc in range(CT)]

            HT = TPG // 2  # 16 k-tiles per staged half-group
            for g in range(G):
                # hs side: group g's 64 emb-rows duplicated into both halves
                hsdup = hsdupp.tile([128, 2 * 128], F16, name="hsdup")
                src_hs = hsT[64 * (g % 2):64 * (g % 2) + 64, g // 2, :]
                nc.scalar.dma_start(hsdup[0:64, :], src_hs)
                nc.scalar.dma_start(hsdup[64:128, :], src_hs)

                for hh in range(2):
                    # ts side: pack (even J | odd J) rows into partition 0,
                    # then broadcast across the partition halves on GPSIMD.
                    # partition_broadcast only writes partition-0-based
                    # ranges on HW, so the odd half goes to a staging tile
                    # and moves up via 32-part cross-quadrant DVE copies.
                    tsp = tspp.tile([1, 2, HT, 2 * 128], F16, name="tsp")
                    nc.sync.dma_start(
                        tsp[:], bass.AP(tensor=ts_dram.tensor,
                                        offset=(ts_dram.offset + g * 64 * 256
                                                + hh * HT * 2 * 256),
                                        ap=[[0, 1], [256, 2], [2 * 256, HT],
                                            [1, 256]]))
                    tsrep = tsrepp.tile([128, HT, 2 * 128], F16, name="tsrep")
                    tso = tsop.tile([64, HT, 2 * 128], F16, name="tso")
                    nc.gpsimd.partition_broadcast(tsrep[0:64, :, :],
                                                  tsp[:, 0, :, :], channels=64)
                    nc.gpsimd.partition_broadcast(tso[:, :, :],
                                                  tsp[:, 1, :, :], channels=64)
                    nc.vector.tensor_scalar(tsrep[64:96, :, :], tso[0:32, :, :],
                                            0.0, None,
                                            op0=mybir.AluOpType.add)
                    nc.vector.tensor_scalar(tsrep[96:128, :, :], tso[32:64, :, :],
                                            0.0, None,
                                            op0=mybir.AluOpType.add)

                    for t in range(HT):
                        kt = g * TPG + hh * HT + t
                        wx_t = wxp.tile([128, EMB], F16, name="wx_t")
                        nc.sync.dma_start(wx_t[:],
                                          Wx_d.ap()[kt * 128:(kt + 1) * 128, :])
                        bl_t = blp.tile([128, 2 * 128], F16, name="bl_t")
                        nc.vector.tensor_tensor(bl_t[:], hsdup[:],
                                                tsrep[:, t, :],
                                                op=mybir.AluOpType.mult)
                        for oc in range(CT):
                            nc.tensor.matmul(ps_feat[oc][:],
                                             wx_t[:, oc * 128:(oc + 1) * 128],
                                             bl_t[:],
                                             start=(kt == 0),
                                             stop=(kt == KT - 1))

            # ---- phase L: bias, relu, layernorm, classifier ----
            featT = persist.tile([128, CT, 256], F32, name="featT")
            for oc in range(CT):
                nc.scalar.activation(featT[:, oc, :], ps_feat[oc][:],
                                     mybir.ActivationFunctionType.Relu,
                                     bias=bx_t[:, oc:oc + 1], scale=1.0)

            # stats: column sums of x and x^2 over all 768 emb-dims
            ps_s = psg.tile([1, 256], F32, name="gen")
            ps_q = psg.tile([1, 256], F32, name="gen")
            for oc in range(CT):
                sq_t = sqp.tile([128, 256], F32, name="sq_t")
                nc.vector.tensor_tensor(sq_t[:], featT[:, oc, :], featT[:, oc, :],
                                        op=mybir.AluOpType.mult)
                nc.tensor.matmul(ps_s[:], ones_t[:], featT[:, oc, :],
                                 start=(oc == 0), stop=(oc == CT - 1))
                nc.tensor.matmul(ps_q[:], ones_t[:], sq_t[:],
                                 start=(oc == 0), stop=(oc == CT - 1))

            mr = smallp.tile([1, 2, 256], F32, name="mr")
            nc.scalar.activation(mr[:, 0, :], ps_s[:1, :],
                                 mybir.ActivationFunctionType.Copy,
                                 bias=0.0, scale=1.0 / EMB)
            exq = smallp.tile([1, 256], F32, name="exq")
            nc.scalar.activation(exq[:], ps_q[:1, :],
                                 mybir.ActivationFunctionType.Copy,
                                 bias=0.0, scale=1.0 / EMB)
            m2 = smallp.tile([1, 256], F32, name="m2")
            nc.vector.tensor_tensor(m2[:], mr[:, 0, :], mr[:, 0, :],
                                    op=mybir.AluOpType.mult)
            var_t = smallp.tile([1, 256], F32, name="var_t")
            nc.vector.tensor_tensor(var_t[:], exq[:], m2[:],
                                    op=mybir.AluOpType.subtract)
            sd_t = smallp.tile([1, 256], F32, name="sd_t")
            nc.scalar.activation(sd_t[:], var_t[:],
                                 mybir.ActivationFunctionType.Sqrt,
                                 bias=eps_t[0:1, :], scale=1.0)
            nc.vector.reciprocal(mr[:, 1, :], sd_t[:])

            # broadcast mean & rstd to all partitions
            mrb = persist.tile([128, 2, 256], F32, name="mrb")
            nc.gpsimd.partition_broadcast(mrb[:], mr[:], channels=128)

            # normalize: ln = (feat - mean) * rstd * lng + lnb
            ln = persist.tile([128, CT, 256], F32, name="ln")
            for oc in range(CT):
                t1 = sqp.tile([128, 256], F32, name="t1")
                nc.vector.tensor_tensor(t1[:], featT[:, oc, :], mrb[:, 0, :],
                                        op=mybir.AluOpType.subtract)
                nc.vector.tensor_tensor(t1[:], t1[:], mrb[:, 1, :],
                                        op=mybir.AluOpType.mult)
                nc.vector.tensor_scalar(ln[:, oc, :], t1[:],
                                        lng_t[:, oc:oc + 1],
                                        lnb_t[:, oc:oc + 1],
                                        op0=mybir.AluOpType.mult,
                                        op1=mybir.AluOpType.add)

            # classifier: logits[pair, 97] = sum_e ln[e, pair] * Wc[e, :]
            for pt in range(2):
                ps_lg = psg.tile([128, NL], F32, name="gen")
                for oc in range(CT):
                    nc.tensor.matmul(ps_lg[:],
                                     ln[:, oc, pt * 128:(pt + 1) * 128],
                                     wc_t[:, oc, :],
                                     start=(oc == 0), stop=(oc == CT - 1))
                out_sb = smallp.tile([128, NL], F32, name="out_sb")
                nc.scalar.copy(out_sb[:], ps_lg[:])
                nc.scalar.dma_start(out_d.ap()[pt * 128:(pt + 1) * 128, :],
                                    out_sb[:])

    nc.compile()
    return nc


_NC_CACHE = []


def _get_module():
    if not _NC_CACHE:
        _NC_CACHE.append(_build_module())
    return _NC_CACHE[0]


def _build_inputs(seq, starts, ends, mention_mask, W_head, b_head, W_tail, b_tail,
                  W_ext, b_ext, ln_g, ln_b, W_cls):
    seq = np.asarray(seq, np.float32)
    starts = np.asarray(starts, np.int64)
    ends = np.asarray(ends, np.int64)
    mask = np.asarray(mention_mask, np.float32)

    # per-document entity selection matrix: ent = Sb^T @ seq[b]
    S_b = np.zeros((B, L, E), np.float32)
    denom = np.maximum(mask.sum(axis=2), 1.0)          # [B, E]
    w = mask * 0.5 / denom[:, :, None]                 # [B, E, M]
    for b in range(B):
        for e in range(E):
            np.add.at(S_b[b, :, e], starts[b, e] + 1, w[b, e])
            np.add.at(S_b[b, :, e], ends[b, e], w[b, e])

    cls_col = np.zeros((L, 1), np.float32)
    cls_col[0, 0] = 1.0

    # W_ext rows (g, i, j) -> (g, t, jh, i) with j = 2t + jh so each
    # 128-row k-tile holds (jh, i) = partition jh*64 + i.
    Wx = np.asarray(W_ext, np.float32).astype(np.float16)
    Wx = Wx.reshape(G, BLK, TPG, 2, EMB).transpose(0, 2, 3, 1, 4)
    Wx = np.ascontiguousarray(Wx.reshape(EMB * BLK, EMB))

    def chunked(v):
        return np.ascontiguousarray(np.asarray(v, np.float32).reshape(CT, 128).T)

    shared = {
        "Wh": np.ascontiguousarray(np.asarray(W_head, np.float32).astype(np.float16)),
        "Wt": np.ascontiguousarray(np.asarray(W_tail, np.float32).astype(np.float16)),
        "bh": chunked(b_head),
        "bt": chunked(b_tail),
        "Wx": Wx,
        "bx": chunked(b_ext),
        "lng": chunked(ln_g),
        "lnb": chunked(ln_b),
        "Wc": np.ascontiguousarray(W_cls, dtype=np.float32),
    }
    in_maps = []
    for core in range(N_CORES):
        b, ib = core // 4, core % 4
        S_core = np.concatenate(
            [S_b[b][:, ib * IB:(ib + 1) * IB], S_b[b], cls_col], axis=1)
        in_maps.append({
            "seq": np.ascontiguousarray(seq[b].astype(np.float16)),
            "S": np.ascontiguousarray(S_core.astype(np.float16)),
            **shared,
        })
    return in_maps


def kernel(**inputs) -> np.ndarray:
    nc = _get_module()
    in_maps = _build_inputs(**inputs)
    res = run_bass_kernel_spmd(nc, in_maps, core_ids=list(range(N_CORES)))
    outs = np.stack([res.results[c]["out"] for c in range(N_CORES)])  # [8,256,97]
    return outs.reshape(B, 4, IB, E, NL).reshape(B, E, E, NL)


# revision 20
# speedup vs baseline: 1.4691x; 1.4691x over previous
"""DocRE model kernel for 8 Trainium2 NeuronCores.

Data-parallel over the pair grid: core = b*4 + ib owns document b and
i-rows [8*ib, 8*ib+8) of the 32x32 entity-pair grid (256 pairs/core).
All weights are replicated; W_ext (49152x768) is streamed from HBM
through a float32r matmul with the group-bilinear feature tiles
materialized on-chip.
"""

import numpy as np

import concourse.bacc as bacc
import concourse.bass as bass
import concourse.tile as tile
from concourse import mybir
from concourse.bass_utils import run_bass_kernel_spmd
from concourse.masks import make_identity

F32 = mybir.dt.float32
F32R = mybir.dt.float32r
F16 = mybir.dt.float16

B, L, H = 2, 1024, 768
E, M = 32, 4
EMB, BLK, NL = 768, 64, 97
G = EMB // BLK  # 12
LN_EPS = 1e-12

N_CORES = 8
IB = E // (N_CORES // B)     # 8 i-rows per core
NPAIR = IB * E               # 256 pairs per core
PT = NPAIR // 128            # 2 pair-tiles
KT = EMB * BLK // 128        # 384 k-tiles
CT = EMB // 128              # 6 feature chunks
KC = H // 128                # 6 contraction chunks of H
LC = L // 128                # 8 chunks of L
NENT = IB + E + 1            # 41 cols: [my 8 entities | all 32 | cls]


def _build_module():
    nc = bacc.Bacc("TRN2", target_bir_lowering=False, debug=False)

    seq_d = nc.dram_tensor("seq", [L, H], F16, kind="ExternalInput")
    S_d = nc.dram_tensor("S", [L, NENT], F16, kind="ExternalInput")
    Wh_d = nc.dram_tensor("Wh", [3 * H, EMB], F16, kind="ExternalInput")
    Wt_d = nc.dram_tensor("Wt", [3 * H, EMB], F16, kind="ExternalInput")
    bh_d = nc.dram_tensor("bh", [128, CT], F32, kind="ExternalInput")
    bt_d = nc.dram_tensor("bt", [128, CT], F32, kind="ExternalInput")
    Wx_d = nc.dram_tensor("Wx", [EMB * BLK, EMB], F16, kind="ExternalInput")
    Ebc_d = nc.dram_tensor("Ebc", [2, 128], F16, kind="ExternalInput")
    bx_d = nc.dram_tensor("bx", [128, EMB], F32, kind="ExternalInput")
    lng_d = nc.dram_tensor("lng", [128, EMB], F32, kind="ExternalInput")
    lnb_d = nc.dram_tensor("lnb", [128, EMB], F32, kind="ExternalInput")
    Wc_d = nc.dram_tensor("Wc", [EMB, NL], F32, kind="ExternalInput")
    out_d = nc.dram_tensor("out", [NPAIR, NL], F32, kind="ExternalOutput")

    with tile.TileContext(nc) as tc:
        with (
            tc.tile_pool(name="persist", bufs=1) as persist,
            tc.tile_pool(name="seqp", bufs=1) as seqp,
            tc.tile_pool(name="whp", bufs=9) as whp,
            tc.tile_pool(name="wxp", bufs=14) as wxp,
            tc.tile_pool(name="blp", bufs=8) as blp,
            tc.tile_pool(name="hsgp", bufs=4) as hsgp,
            tc.tile_pool(name="tmpp", bufs=3) as tmpp,
            tc.tile_pool(name="dramp", bufs=1, space="DRAM") as dramp,
            tc.tile_pool(name="psf", bufs=1, space="PSUM") as psf,
            tc.tile_pool(name="psg", bufs=2, space="PSUM") as psg,
            tc.tile_pool(name="psb", bufs=2, space="PSUM") as psb,
        ):
            ident = persist.tile([128, 128], F32, name="ident")
            make_identity(nc, ident[:])
            E_t = persist.tile([2, 128], F16, name="E_t")
            nc.sync.dma_start(E_t[:], Ebc_d.ap())

            # ---- per-column constants broadcast to all partitions ----
            bx_b = persist.tile([128, EMB], F32, name="bx_b")
            lng_b = persist.tile([128, EMB], F32, name="lng_b")
            lnb_b = persist.tile([128, EMB], F32, name="lnb_b")
            for tile_, src in ((bx_b, bx_d), (lng_b, lng_d), (lnb_b, lnb_d)):
                nc.sync.dma_start(tile_[:], src.ap())

            eps_t = persist.tile([128, 1], F32, name="eps")
            nc.vector.memset(eps_t[:], LN_EPS)

            # per-partition bias chunks bh/bt: [128, CT]
            bh_t = persist.tile([128, CT], F32, name="bh_t")
            bt_t = persist.tile([128, CT], F32, name="bt_t")
            for tile_, src in ((bh_t, bh_d), (bt_t, bt_d)):
                nc.sync.dma_start(tile_[:], src.ap())

            # ---- phase E: entity pooling  ent = S^T @ seq ----
            seq_t = seqp.tile([128, LC, H], F16, name="seq_t")
            S_t = seqp.tile([128, LC, NENT], F16, name="S_t")
            seq_re = seq_d.ap().rearrange("(c p) h -> p c h", p=128)
            S_re = S_d.ap().rearrange("(c p) n -> p c n", p=128)
            for kc in range(LC):
                nc.sync.dma_start(S_t[:, kc, :], S_re[:, kc, :])
                nc.sync.dma_start(seq_t[:, kc, :], seq_re[:, kc, :])

            ps_e0 = psg.tile([NENT, 512], F32, name="gen")
            ps_e1 = psg.tile([NENT, 256], F32, name="gen")
            for kc in range(LC):
                nc.tensor.matmul(ps_e0[:], S_t[:, kc, :], seq_t[:, kc, 0:512],
                                 start=(kc == 0), stop=(kc == LC - 1))
                nc.tensor.matmul(ps_e1[:], S_t[:, kc, :], seq_t[:, kc, 512:768],
                                 start=(kc == 0), stop=(kc == LC - 1))
            ent_nat = persist.tile([NENT, H], F32, name="ent_nat")
            nc.scalar.copy(ent_nat[:, 0:512], ps_e0[:])
            nc.scalar.copy(ent_nat[:, 512:768], ps_e1[:])

            # transpose ent -> entT [h, NENT]  (f32r: feeds phase-A matmuls)
            entT = persist.tile([128, KC, NENT], F16, name="entT")
            for kc in range(KC):
                ps_tr = psg.tile([128, NENT], F32, name="gen")
                nc.tensor.transpose(ps_tr[:], ent_nat[:, kc * 128:(kc + 1) * 128],
                                    ident[:NENT, :NENT])
                nc.scalar.copy(entT[:, kc, :], ps_tr[:])

            # ---- phase A: A/B/C projections ----
            # natural layout first: X_nat = ent @ W_block  [41, 768], then
            # PE-transpose into ABCD[ct][:, m, :] ([c,41], m: Ah,Bh,At,Bt).
            ABCD = []
            for ct in range(CT):
                abcd_alloc = persist.tile([128, 4, NENT + 1], F32, name=f"abcd{ct}")
                nc.vector.memset(abcd_alloc[:], 0.0)
                ABCD.append(abcd_alloc)

            # feat accumulators allocated early: phase-A chains borrow these
            # idle PSUM banks so three chains can run concurrently.
            ps_feat = [[psf.tile([128, 512], F32, name=f"pf{pt}a"),
                        psf.tile([128, 256], F32, name=f"pf{pt}b")]
                       for pt in range(PT)]

            def emit_ab_chain(m, w_d, blk, ps_pair=None):
                if ps_pair is None:
                    ps_n0 = psg.tile([NENT, 512], F32, name="gen")
                    ps_n1 = psg.tile([NENT, 256], F32, name="gen")
                else:
                    ps_n0 = ps_pair[0][:NENT, :]
                    ps_n1 = ps_pair[1][:NENT, :]
                for kc in range(KC):
                    w_t = whp.tile([128, EMB], F16, name="w_t")
                    nc.sync.dma_start(
                        w_t[:], w_d.ap()[blk * H + kc * 128: blk * H + (kc + 1) * 128, :])
                    nc.tensor.matmul(ps_n0[:], entT[:, kc, :], w_t[:, 0:512],
                                     start=(kc == 0), stop=(kc == KC - 1))
                    nc.tensor.matmul(ps_n1[:], entT[:, kc, :], w_t[:, 512:768],
                                     start=(kc == 0), stop=(kc == KC - 1))
                x_nat = tmpp.tile([NENT, EMB], F32, name="x_nat")
                nc.scalar.copy(x_nat[:, 0:512], ps_n0[:])
                nc.scalar.copy(x_nat[:, 512:768], ps_n1[:])
                for ct in range(CT):
                    ps_tr = psg.tile([128, NENT], F32, name="gen")
                    nc.tensor.transpose(ps_tr[:], x_nat[:, ct * 128:(ct + 1) * 128],
                                        ident[:NENT, :NENT])
                    nc.scalar.copy(ABCD[ct][:, m, 0:NENT], ps_tr[:])

            def emit_c_chain(m_sel, w_d, bias_t):
                ps_c0 = psg.tile([NENT, 512], F32, name="gen")
                ps_c1 = psg.tile([NENT, 256], F32, name="gen")
                for kc in range(KC):
                    w_t = whp.tile([128, EMB], F16, name="w_t")
                    nc.sync.dma_start(
                        w_t[:], w_d.ap()[2 * H + kc * 128: 2 * H + (kc + 1) * 128, :])
                    nc.tensor.matmul(ps_c0[:1, :], entT[:, kc, IB + E:IB + E + 1],
                                     w_t[:, 0:512],
                                     start=(kc == 0), stop=(kc == KC - 1))
                    nc.tensor.matmul(ps_c1[:1, :], entT[:, kc, IB + E:IB + E + 1],
                                     w_t[:, 512:768],
                                     start=(kc == 0), stop=(kc == KC - 1))
                c_nat = tmpp.tile([1, EMB], F32, name="c_nat")
                nc.scalar.copy(c_nat[:, 0:512], ps_c0[:1, :])
                nc.scalar.copy(c_nat[:, 512:768], ps_c1[:1, :])
                for ct in range(CT):
                    ps_tr = psg.tile([128, NENT], F32, name="gen")
                    nc.tensor.transpose(ps_tr[:, 0:1],
                                        c_nat[:, ct * 128:(ct + 1) * 128],
                                        ident[:1, :1])
                    nc.vector.tensor_tensor(ABCD[ct][:, m_sel, NENT:NENT + 1],
                                            ps_tr[:, 0:1],
                                            bias_t[:, ct:ct + 1],
                                            op=mybir.AluOpType.add)

            emit_c_chain(3, Wt_d, bt_t)
            emit_ab_chain(2, Wt_d, 0, ps_feat[0])
            emit_ab_chain(3, Wt_d, 1, ps_feat[1])


            # ---- phase P ts-side: tsdup generated from duplicated ABCD ----
            # col = pt*128 + il*32 + j ; i = 8*ib + pt*4 + il
            hsT = persist.tile([128, CT, 2 * 128], F16, name="hsT")
            tsdup = persist.tile([128, G, 2 * 128], F16, name="tsdup")
            hs_dram = dramp.tile([EMB, 2 * 128], F16, name="hs_dram")
            NE2 = NENT + 1

            def colview(tile_, m, col0, ap_pat):
                return bass.AP(tensor=tile_.tensor,
                               offset=tile_.offset + m * NE2 + col0,
                               ap=[tile_.ap[0]] + ap_pat)

            for ct in range(CT):
                abcd_t = ABCD[ct]
                for half in range(2):
                    g = 2 * ct + half
                    dup_t = tmpp.tile([128, 4, NE2], F32, name="dup")
                    src_ab = abcd_t[half * 64:half * 64 + 64, :, :]
                    nc.scalar.dma_start(dup_t[0:64, :, :], src_ab)
                    nc.scalar.dma_start(dup_t[64:128, :, :], src_ab)
                    tmp2 = tmpp.tile([128, 8, 32], F32, name="tmp")
                    nc.vector.tensor_tensor(
                        tmp2[:], colview(dup_t, 2, IB, [[0, 8], [1, 32]]),
                        colview(dup_t, 3, 0, [[1, 8], [0, 32]]),
                        op=mybir.AluOpType.add)
                    nc.scalar.activation(
                        tsdup[:, g, :].rearrange("p (a b) -> p a b", a=8),
                        tmp2[:], mybir.ActivationFunctionType.Tanh,
                        bias=dup_t[:, 3, NENT:NENT + 1], scale=1.0)

            # ---- head-side projections, then hs generation ----
            emit_c_chain(0, Wh_d, bh_t)
            emit_ab_chain(0, Wh_d, 0, ps_feat[0])
            emit_ab_chain(1, Wh_d, 1, ps_feat[1])
            for ct in range(CT):
                abcd_t = ABCD[ct]
                tmp = tmpp.tile([128, 8, 32], F32, name="tmp")
                nc.vector.tensor_tensor(
                    tmp[:], colview(abcd_t, 0, 0, [[1, 8], [0, 32]]),
                    colview(abcd_t, 1, IB, [[0, 8], [1, 32]]),
                    op=mybir.AluOpType.add)
                nc.scalar.activation(
                    hsT[:, ct, :].rearrange("p (a b) -> p a b", a=8),
                    tmp[:], mybir.ActivationFunctionType.Tanh,
                    bias=abcd_t[:, 0, NENT:NENT + 1], scale=1.0)
                nc.scalar.dma_start(hs_dram[ct * 128:(ct + 1) * 128, :],
                                    hsT[:, ct, :])

            # ---- phase M: main contraction over W_ext ----
            # software pipeline: broadcasts for upcoming k-tiles issue before
            # the current k-tiles' main matmuls so the DVE multiply latency is
            # hidden behind PE work.  Broadcast matmuls are emitted two at a
            # time so they share one E_t LDWEIGHTS; main matmuls stream
            # 256-col chunks (measured faster per column than 512-col).
            hsg_tiles = {}

            def stage_group(g):
                # stage group g's 64 hs rows into partitions 0-1, pair-major:
                # hsg[r, tq, 256*q + p] = hs row (g*64 + 4*tq + 2*q + r)
                hsg_tiles[g] = hsgp.tile([2, 16, 512], F16, name="hsg")
                nc.scalar.dma_start(
                    hsg_tiles[g][:].rearrange("r tq (q p) -> r tq q p", q=2),
                    bass.AP(tensor=hs_dram.tensor,
                            offset=hs_dram.offset + g * 64 * 2 * 128,
                            ap=[[256, 2], [4 * 256, 16], [2 * 256, 2], [1, 256]]))

            def emit_bc_quad(ktq):
                # two [2,512] rhs matmuls sharing one E_t weight load;
                # covers k-tiles 4*ktq .. 4*ktq+3
                kt0 = 4 * ktq
                g = kt0 // 32
                tq = (kt0 % 32) // 2
                if kt0 % 32 == 24 and g + 1 < G:
                    stage_group(g + 1)
                bc_a = psb.tile([128, 512], F32, name="bc_ps")
                bc_b = psb.tile([128, 512], F32, name="bc_ps")
                nc.tensor.matmul(bc_a[:], E_t[:], hsg_tiles[g][:, tq, :],
                                 start=True, stop=True)
                nc.tensor.matmul(bc_b[:], E_t[:], hsg_tiles[g][:, tq + 1, :],
                                 start=True, stop=True)
                return bc_a, bc_b

            stage_group(0)
            bc_cur = emit_bc_quad(0)
            for kt2 in range(KT // 2):
                wx_t = wxp.tile([128, 2, EMB], F16, name="wx_t")
                nc.sync.dma_start(
                    wx_t[:],
                    Wx_d.ap()[kt2 * 256:(kt2 + 1) * 256, :]
                    .rearrange("(q p) e -> p q e", q=2))
                for q in range(2):
                    kt = 2 * kt2 + q
                    g = kt // 32
                    bc_ps = bc_cur[kt2 % 2]
                    bl_t = blp.tile([128, 2 * 128], F16, name="bl_t")
                    nc.vector.tensor_tensor(bl_t[:],
                                            bc_ps[:, q * 256:(q + 1) * 256],
                                            tsdup[:, g, :],
                                            op=mybir.AluOpType.mult)
                    if kt % 4 == 3 and kt + 1 < KT:
                        bc_cur = emit_bc_quad((kt + 1) // 4)
                    first, last = (kt == 0), (kt == KT - 1)
                    for pt in range(PT):
                        lhsT = bl_t[:, pt * 128:(pt + 1) * 128]
                        if first or last:
                            nc.tensor.matmul(ps_feat[pt][0][:], lhsT,
                                             wx_t[:, q, 0:512],
                                             start=first, stop=last)
                        else:
                            nc.tensor.matmul(ps_feat[pt][0][:, 0:256], lhsT,
                                             wx_t[:, q, 0:256],
                                             start=False, stop=False)
                            nc.tensor.matmul(ps_feat[pt][0][:, 256:512], lhsT,
                                             wx_t[:, q, 256:512],
                                             start=False, stop=False)
                        nc.tensor.matmul(ps_feat[pt][1][:], lhsT,
                                         wx_t[:, q, 512:768],
                                         start=first, stop=last)

            # ---- phase L: bias, relu, layernorm, classifier ----
            wc_t = persist.tile([128, CT, NL], F32, name="wc_t")
            nc.sync.dma_start(wc_t[:], Wc_d.ap().rearrange("(c p) n -> p c n", p=128))

            for pt in range(PT):
                feat = persist.tile([128, EMB], F32, name=f"feat{pt}")
                nc.vector.tensor_tensor(feat[:, 0:512], ps_feat[pt][0][:],
                                        bx_b[:, 0:512], op=mybir.AluOpType.add)
                nc.vector.tensor_tensor(feat[:, 512:768], ps_feat[pt][1][:],
                                        bx_b[:, 512:768], op=mybir.AluOpType.add)
                nc.scalar.activation(feat[:], feat[:],
                                     mybir.ActivationFunctionType.Relu,
                                     bias=0.0, scale=1.0)

                stats = tmpp.tile([128, 3, 6], F32, name="stats")
                f_re = feat.rearrange("p (c f) -> p c f", c=3)
                for c in range(3):
                    nc.vector.bn_stats(stats[:, c, :], f_re[:, c, :])
                mv = tmpp.tile([128, 2], F32, name="mv")
                nc.vector.bn_aggr(mv[:], stats[:])
                sd = tmpp.tile([128, 1], F32, name="sd")
                nc.scalar.activation(sd[:], mv[:, 1:2],
                                     mybir.ActivationFunctionType.Sqrt,
                                     bias=eps_t[:], scale=1.0)
                rstd = tmpp.tile([128, 1], F32, name="rstd")
                nc.vector.reciprocal(rstd[:], sd[:])

                ln = persist.tile([128, EMB], F32, name=f"ln{pt}")
                nc.vector.tensor_scalar(ln[:], feat[:], mv[:, 0:1], rstd[:],
                                        op0=mybir.AluOpType.subtract,
                                        op1=mybir.AluOpType.mult)
                nc.vector.tensor_tensor(ln[:], ln[:], lng_b[:],
                                        op=mybir.AluOpType.mult)
                nc.vector.tensor_tensor(ln[:], ln[:], lnb_b[:],
                                        op=mybir.AluOpType.add)

                lnT = persist.tile([128, CT, 128], F32, name=f"lnT{pt}")
                for ct in range(CT):
                    ps_tr2 = psg.tile([128, 128], F32, name="gen")
                    nc.tensor.transpose(ps_tr2[:], ln[:, ct * 128:(ct + 1) * 128],
                                        ident[:])
                    nc.scalar.copy(lnT[:, ct, :], ps_tr2[:])

                ps_lg = psg.tile([128, NL], F32, name="gen")
                for ct in range(CT):
                    nc.tensor.matmul(ps_lg[:], lnT[:, ct, :], wc_t[:, ct, :],
                                     start=(ct == 0), stop=(ct == CT - 1))
                out_sb = tmpp.tile([128, NL], F32, name="out_sb")
                nc.scalar.copy(out_sb[:], ps_lg[:])
                nc.scalar.dma_start(out_d.ap()[pt * 128:(pt + 1) * 128, :], out_sb[:])

    nc.compile()
    return nc


_NC_CACHE = []


def _get_module():
    if not _NC_CACHE:
        _NC_CACHE.append(_build_module())
    return _NC_CACHE[0]


_EBC = np.zeros((2, 128), np.float16)
_EBC[0, :64] = 1.0
_EBC[1, 64:] = 1.0


def _build_inputs(seq, starts, ends, mention_mask, W_head, b_head, W_tail, b_tail,
                  W_ext, b_ext, ln_g, ln_b, W_cls):
    seq = np.asarray(seq, np.float32)
    starts = np.asarray(starts, np.int64)
    ends = np.asarray(ends, np.int64)
    mask = np.asarray(mention_mask, np.float32)

    # per-document entity selection matrix: ent = Sb^T @ seq[b]
    S_b = np.zeros((B, L, E), np.float32)
    denom = np.maximum(mask.sum(axis=2), 1.0)          # [B, E]
    w = mask * 0.5 / denom[:, :, None]                 # [B, E, M]
    for b in range(B):
        for e in range(E):
            np.add.at(S_b[b, :, e], starts[b, e] + 1, w[b, e])
            np.add.at(S_b[b, :, e], ends[b, e], w[b, e])

    cls_col = np.zeros((L, 1), np.float32)
    cls_col[0, 0] = 1.0

    shared = {
        "Wh": np.ascontiguousarray(np.asarray(W_head, np.float32).astype(np.float16)),
        "Wt": np.ascontiguousarray(np.asarray(W_tail, np.float32).astype(np.float16)),
        "bh": np.ascontiguousarray(np.asarray(b_head, np.float32).reshape(CT, 128).T),
        "bt": np.ascontiguousarray(np.asarray(b_tail, np.float32).reshape(CT, 128).T),
        "Wx": np.ascontiguousarray(np.asarray(W_ext).astype(np.float16)),
        "Ebc": _EBC,
        "bx": np.ascontiguousarray(np.broadcast_to(np.asarray(b_ext, np.float32), (128, EMB))),
        "lng": np.ascontiguousarray(np.broadcast_to(np.asarray(ln_g, np.float32), (128, EMB))),
        "lnb": np.ascontiguousarray(np.broadcast_to(np.asarray(ln_b, np.float32), (128, EMB))),
        "Wc": np.ascontiguousarray(W_cls, dtype=np.float32),
    }
    in_maps = []
    for core in range(N_CORES):
        b, ib = core // 4, core % 4
        S_core = np.concatenate(
            [S_b[b][:, ib * IB:(ib + 1) * IB], S_b[b], cls_col], axis=1)
        in_maps.append({
            "seq": np.ascontiguousarray(seq[b].astype(np.float16)),
            "S": np.ascontiguousarray(S_core.astype(np.float16)),
            **shared,
        })
    return in_maps


def kernel(**inputs) -> np.ndarray:
    nc = _get_module()
    in_maps = _build_inputs(**inputs)
    res = run_bass_kernel_spmd(nc, in_maps, core_ids=list(range(N_CORES)))
    outs = np.stack([res.results[c]["out"] for c in range(N_CORES)])  # [8,256,97]
    return outs.reshape(B, 4, IB, E, NL).reshape(B, E, E, NL)



# revision 24
# speedup vs baseline: 1.4716x; 1.0017x over previous
"""DocRE model kernel for 8 Trainium2 NeuronCores.

Data-parallel over the pair grid: core = b*4 + ib owns document b and
i-rows [8*ib, 8*ib+8) of the 32x32 entity-pair grid (256 pairs/core).
All weights are replicated; W_ext (49152x768) is streamed from HBM
through a float32r matmul with the group-bilinear feature tiles
materialized on-chip.
"""

import numpy as np

import concourse.bacc as bacc
import concourse.bass as bass
import concourse.tile as tile
from concourse import mybir
from concourse.bass_utils import run_bass_kernel_spmd
from concourse.masks import make_identity

F32 = mybir.dt.float32
F32R = mybir.dt.float32r
F16 = mybir.dt.float16

B, L, H = 2, 1024, 768
E, M = 32, 4
EMB, BLK, NL = 768, 64, 97
G = EMB // BLK  # 12
LN_EPS = 1e-12

N_CORES = 8
IB = E // (N_CORES // B)     # 8 i-rows per core
NPAIR = IB * E               # 256 pairs per core
PT = NPAIR // 128            # 2 pair-tiles
KT = EMB * BLK // 128        # 384 k-tiles
CT = EMB // 128              # 6 feature chunks
KC = H // 128                # 6 contraction chunks of H
LC = L // 128                # 8 chunks of L
NENT = IB + E + 1            # 41 cols: [my 8 entities | all 32 | cls]


def _build_module():
    nc = bacc.Bacc("TRN2", target_bir_lowering=False, debug=False)

    seq_d = nc.dram_tensor("seq", [L, H], F16, kind="ExternalInput")
    S_d = nc.dram_tensor("S", [L, NENT], F16, kind="ExternalInput")
    Wh_d = nc.dram_tensor("Wh", [3 * H, EMB], F16, kind="ExternalInput")
    Wt_d = nc.dram_tensor("Wt", [3 * H, EMB], F16, kind="ExternalInput")
    bh_d = nc.dram_tensor("bh", [128, CT], F32, kind="ExternalInput")
    bt_d = nc.dram_tensor("bt", [128, CT], F32, kind="ExternalInput")
    Wx_d = nc.dram_tensor("Wx", [EMB * BLK, EMB], F16, kind="ExternalInput")
    Ebc_d = nc.dram_tensor("Ebc", [2, 128], F16, kind="ExternalInput")
    bx_d = nc.dram_tensor("bx", [128, EMB], F32, kind="ExternalInput")
    lng_d = nc.dram_tensor("lng", [128, EMB], F32, kind="ExternalInput")
    lnb_d = nc.dram_tensor("lnb", [128, EMB], F32, kind="ExternalInput")
    Wc_d = nc.dram_tensor("Wc", [EMB, NL], F32, kind="ExternalInput")
    out_d = nc.dram_tensor("out", [NPAIR, NL], F32, kind="ExternalOutput")

    with tile.TileContext(nc) as tc:
        with (
            tc.tile_pool(name="persist", bufs=1) as persist,
            tc.tile_pool(name="seqp", bufs=1) as seqp,
            tc.tile_pool(name="whp", bufs=12) as whp,
            tc.tile_pool(name="wxp", bufs=10) as wxp,
            tc.tile_pool(name="blp", bufs=8) as blp,
            tc.tile_pool(name="hsgp", bufs=4) as hsgp,
            tc.tile_pool(name="tmpp", bufs=3) as tmpp,
            tc.tile_pool(name="dramp", bufs=1, space="DRAM") as dramp,
            tc.tile_pool(name="psf", bufs=1, space="PSUM") as psf,
            tc.tile_pool(name="psg", bufs=2, space="PSUM") as psg,
            tc.tile_pool(name="psb", bufs=2, space="PSUM") as psb,
        ):
            ident = persist.tile([128, 128], F32, name="ident")
            make_identity(nc, ident[:])
            E_t = persist.tile([2, 128], F16, name="E_t")
            nc.sync.dma_start(E_t[:], Ebc_d.ap())

            # ---- per-column constants broadcast to all partitions ----
            # (allocated here; the 1.2MB of DMAs are issued after phase P so
            # they don't delay the phase E/A input streams at the head)
            bx_b = persist.tile([128, EMB], F32, name="bx_b")
            lng_b = persist.tile([128, EMB], F32, name="lng_b")
            lnb_b = persist.tile([128, EMB], F32, name="lnb_b")

            eps_t = persist.tile([128, 1], F32, name="eps")
            nc.vector.memset(eps_t[:], LN_EPS)

            # per-partition bias chunks bh/bt: [128, CT]
            bh_t = persist.tile([128, CT], F32, name="bh_t")
            bt_t = persist.tile([128, CT], F32, name="bt_t")
            for tile_, src in ((bh_t, bh_d), (bt_t, bt_d)):
                nc.sync.dma_start(tile_[:], src.ap())

            # ---- phase E: entity pooling  ent = S^T @ seq ----
            seq_t = seqp.tile([128, LC, H], F16, name="seq_t")
            S_t = seqp.tile([128, LC, NENT], F16, name="S_t")
            seq_re = seq_d.ap().rearrange("(c p) h -> p c h", p=128)
            S_re = S_d.ap().rearrange("(c p) n -> p c n", p=128)
            for kc in range(LC):
                nc.sync.dma_start(S_t[:, kc, :], S_re[:, kc, :])
                nc.sync.dma_start(seq_t[:, kc, :], seq_re[:, kc, :])

            ps_e0 = psg.tile([NENT, 512], F32, name="gen")
            ps_e1 = psg.tile([NENT, 256], F32, name="gen")
            for kc in range(LC):
                nc.tensor.matmul(ps_e0[:], S_t[:, kc, :], seq_t[:, kc, 0:512],
                                 start=(kc == 0), stop=(kc == LC - 1))
                nc.tensor.matmul(ps_e1[:], S_t[:, kc, :], seq_t[:, kc, 512:768],
                                 start=(kc == 0), stop=(kc == LC - 1))
            ent_nat = persist.tile([NENT, H], F32, name="ent_nat")
            nc.scalar.copy(ent_nat[:, 0:512], ps_e0[:])
            nc.scalar.copy(ent_nat[:, 512:768], ps_e1[:])

            # transpose ent -> entT [h, NENT]  (f32r: feeds phase-A matmuls)
            entT = persist.tile([128, KC, NENT], F16, name="entT")
            for kc in range(KC):
                ps_tr = psg.tile([128, NENT], F32, name="gen")
                nc.tensor.transpose(ps_tr[:], ent_nat[:, kc * 128:(kc + 1) * 128],
                                    ident[:NENT, :NENT])
                nc.scalar.copy(entT[:, kc, :], ps_tr[:])

            # ---- phase A: A/B/C projections ----
            # natural layout first: X_nat = ent @ W_block  [41, 768], then
            # PE-transpose into ABCD[ct][:, m, :] ([c,41], m: Ah,Bh,At,Bt).
            ABCD = []
            for ct in range(CT):
                abcd_alloc = persist.tile([128, 4, NENT + 1], F32, name=f"abcd{ct}")
                nc.vector.memset(abcd_alloc[:], 0.0)
                ABCD.append(abcd_alloc)

            # feat accumulators allocated early: phase-A chains borrow these
            # idle PSUM banks so three chains can run concurrently.
            ps_feat = [[psf.tile([128, 512], F32, name=f"pf{pt}a"),
                        psf.tile([128, 256], F32, name=f"pf{pt}b")]
                       for pt in range(PT)]

            def emit_ab_chain(m, w_d, blk, ps_pair=None):
                if ps_pair is None:
                    ps_n0 = psg.tile([NENT, 512], F32, name="gen")
                    ps_n1 = psg.tile([NENT, 256], F32, name="gen")
                else:
                    ps_n0 = ps_pair[0][:NENT, :]
                    ps_n1 = ps_pair[1][:NENT, :]
                for kc in range(KC):
                    w_t = whp.tile([128, EMB], F16, name="w_t")
                    nc.sync.dma_start(
                        w_t[:], w_d.ap()[blk * H + kc * 128: blk * H + (kc + 1) * 128, :])
                    nc.tensor.matmul(ps_n0[:], entT[:, kc, :], w_t[:, 0:512],
                                     start=(kc == 0), stop=(kc == KC - 1))
                    nc.tensor.matmul(ps_n1[:], entT[:, kc, :], w_t[:, 512:768],
                                     start=(kc == 0), stop=(kc == KC - 1))
                x_nat = tmpp.tile([NENT, EMB], F32, name="x_nat")
                nc.scalar.copy(x_nat[:, 0:512], ps_n0[:])
                nc.scalar.copy(x_nat[:, 512:768], ps_n1[:])
                for ct in range(CT):
                    ps_tr = psg.tile([128, NENT], F32, name="gen")
                    nc.tensor.transpose(ps_tr[:], x_nat[:, ct * 128:(ct + 1) * 128],
                                        ident[:NENT, :NENT])
                    nc.scalar.copy(ABCD[ct][:, m, 0:NENT], ps_tr[:])

            def emit_c_chain(m_sel, w_d, bias_t):
                ps_c0 = psg.tile([NENT, 512], F32, name="gen")
                ps_c1 = psg.tile([NENT, 256], F32, name="gen")
                for kc in range(KC):
                    w_t = whp.tile([128, EMB], F16, name="w_t")
                    nc.sync.dma_start(
                        w_t[:], w_d.ap()[2 * H + kc * 128: 2 * H + (kc + 1) * 128, :])
                    nc.tensor.matmul(ps_c0[:1, :], entT[:, kc, IB + E:IB + E + 1],
                                     w_t[:, 0:512],
                                     start=(kc == 0), stop=(kc == KC - 1))
                    nc.tensor.matmul(ps_c1[:1, :], entT[:, kc, IB + E:IB + E + 1],
                                     w_t[:, 512:768],
                                     start=(kc == 0), stop=(kc == KC - 1))
                c_nat = tmpp.tile([1, EMB], F32, name="c_nat")
                nc.scalar.copy(c_nat[:, 0:512], ps_c0[:1, :])
                nc.scalar.copy(c_nat[:, 512:768], ps_c1[:1, :])
                for ct in range(CT):
                    ps_tr = psg.tile([128, NENT], F32, name="gen")
                    nc.tensor.transpose(ps_tr[:, 0:1],
                                        c_nat[:, ct * 128:(ct + 1) * 128],
                                        ident[:1, :1])
                    nc.vector.tensor_tensor(ABCD[ct][:, m_sel, NENT:NENT + 1],
                                            ps_tr[:, 0:1],
                                            bias_t[:, ct:ct + 1],
                                            op=mybir.AluOpType.add)

            emit_c_chain(3, Wt_d, bt_t)
            emit_ab_chain(2, Wt_d, 0, ps_feat[0])
            emit_ab_chain(3, Wt_d, 1, ps_feat[1])


            # ---- phase P ts-side: tsdup generated from duplicated ABCD ----
            # col = pt*128 + il*32 + j ; i = 8*ib + pt*4 + il
            hsT = persist.tile([128, CT, 2 * 128], F16, name="hsT")
            tsdup = persist.tile([128, G, 2 * 128], F16, name="tsdup")
            hs_dram = dramp.tile([EMB, 2 * 128], F16, name="hs_dram")
            NE2 = NENT + 1

            def colview(tile_, m, col0, ap_pat):
                return bass.AP(tensor=tile_.tensor,
                               offset=tile_.offset + m * NE2 + col0,
                               ap=[tile_.ap[0]] + ap_pat)

            for ct in range(CT):
                abcd_t = ABCD[ct]
                for half in range(2):
                    g = 2 * ct + half
                    dup_t = tmpp.tile([128, 4, NE2], F32, name="dup")
                    src_ab = abcd_t[half * 64:half * 64 + 64, :, :]
                    nc.scalar.dma_start(dup_t[0:64, :, :], src_ab)
                    nc.scalar.dma_start(dup_t[64:128, :, :], src_ab)
                    tmp2 = tmpp.tile([128, 8, 32], F32, name="tmp")
                    nc.vector.tensor_tensor(
                        tmp2[:], colview(dup_t, 2, IB, [[0, 8], [1, 32]]),
                        colview(dup_t, 3, 0, [[1, 8], [0, 32]]),
                        op=mybir.AluOpType.add)
                    nc.scalar.activation(
                        tsdup[:, g, :].rearrange("p (a b) -> p a b", a=8),
                        tmp2[:], mybir.ActivationFunctionType.Tanh,
                        bias=dup_t[:, 3, NENT:NENT + 1], scale=1.0)

            # ---- head-side projections, then hs generation ----
            emit_c_chain(0, Wh_d, bh_t)
            emit_ab_chain(0, Wh_d, 0, ps_feat[0])
            emit_ab_chain(1, Wh_d, 1, ps_feat[1])
            for ct in range(CT):
                abcd_t = ABCD[ct]
                tmp = tmpp.tile([128, 8, 32], F32, name="tmp")
                nc.vector.tensor_tensor(
                    tmp[:], colview(abcd_t, 0, 0, [[1, 8], [0, 32]]),
                    colview(abcd_t, 1, IB, [[0, 8], [1, 32]]),
                    op=mybir.AluOpType.add)
                nc.scalar.activation(
                    hsT[:, ct, :].rearrange("p (a b) -> p a b", a=8),
                    tmp[:], mybir.ActivationFunctionType.Tanh,
                    bias=abcd_t[:, 0, NENT:NENT + 1], scale=1.0)
                nc.scalar.dma_start(hs_dram[ct * 128:(ct + 1) * 128, :],
                                    hsT[:, ct, :])

            # phase-L constants: issued now so they queue behind the
            # phase E/A input streams but ahead of the bulk of W_ext.
            for tile_, src in ((bx_b, bx_d), (lng_b, lng_d), (lnb_b, lnb_d)):
                nc.sync.dma_start(tile_[:], src.ap())

            # ---- phase M: main contraction over W_ext ----
            # software pipeline: broadcasts for upcoming k-tiles issue before
            # the current k-tiles' main matmuls so the DVE multiply latency is
            # hidden behind PE work.  Broadcast matmuls are emitted two at a
            # time so they share one E_t LDWEIGHTS; main matmuls stream
            # 256-col chunks (measured faster per column than 512-col).
            hsg_tiles = {}

            def stage_group(g):
                # stage group g's 64 hs rows into partitions 0-1, pair-major:
                # hsg[r, tq, 256*q + p] = hs row (g*64 + 4*tq + 2*q + r)
                hsg_tiles[g] = hsgp.tile([2, 16, 512], F16, name="hsg")
                nc.scalar.dma_start(
                    hsg_tiles[g][:].rearrange("r tq (q p) -> r tq q p", q=2),
                    bass.AP(tensor=hs_dram.tensor,
                            offset=hs_dram.offset + g * 64 * 2 * 128,
                            ap=[[256, 2], [4 * 256, 16], [2 * 256, 2], [1, 256]]))

            def emit_bc_quad(ktq):
                # two [2,512] rhs matmuls sharing one E_t weight load;
                # covers k-tiles 4*ktq .. 4*ktq+3
                kt0 = 4 * ktq
                g = kt0 // 32
                tq = (kt0 % 32) // 2
                if kt0 % 32 == 24 and g + 1 < G:
                    stage_group(g + 1)
                bc_a = psb.tile([128, 512], F32, name="bc_ps")
                bc_b = psb.tile([128, 512], F32, name="bc_ps")
                nc.tensor.matmul(bc_a[:], E_t[:], hsg_tiles[g][:, tq, :],
                                 start=True, stop=True)
                nc.tensor.matmul(bc_b[:], E_t[:], hsg_tiles[g][:, tq + 1, :],
                                 start=True, stop=True)
                return bc_a, bc_b

            stage_group(0)
            bc_cur = emit_bc_quad(0)
            for kt2 in range(KT // 2):
                wx_t = wxp.tile([128, 2, EMB], F16, name="wx_t")
                nc.sync.dma_start(
                    wx_t[:],
                    Wx_d.ap()[kt2 * 256:(kt2 + 1) * 256, :]
                    .rearrange("(q p) e -> p q e", q=2))
                for q in range(2):
                    kt = 2 * kt2 + q
                    g = kt // 32
                    bc_ps = bc_cur[kt2 % 2]
                    bl_t = blp.tile([128, 2 * 128], F16, name="bl_t")
                    nc.vector.tensor_tensor(bl_t[:],
                                            bc_ps[:, q * 256:(q + 1) * 256],
                                            tsdup[:, g, :],
                                            op=mybir.AluOpType.mult)
                    if kt % 4 == 3 and kt + 1 < KT:
                        bc_cur = emit_bc_quad((kt + 1) // 4)
                    first, last = (kt == 0), (kt == KT - 1)
                    for pt in range(PT):
                        lhsT = bl_t[:, pt * 128:(pt + 1) * 128]
                        if first or last:
                            nc.tensor.matmul(ps_feat[pt][0][:], lhsT,
                                             wx_t[:, q, 0:512],
                                             start=first, stop=last)
                        else:
                            nc.tensor.matmul(ps_feat[pt][0][:, 0:256], lhsT,
                                             wx_t[:, q, 0:256],
                                             start=False, stop=False)
                            nc.tensor.matmul(ps_feat[pt][0][:, 256:512], lhsT,
                                             wx_t[:, q, 256:512],
                                             start=False, stop=False)
                        nc.tensor.matmul(ps_feat[pt][1][:], lhsT,
                                         wx_t[:, q, 512:768],
                                         start=first, stop=last)

            # ---- phase L: bias, relu, layernorm, classifier ----
            wc_t = persist.tile([128, CT, NL], F32, name="wc_t")
            nc.sync.dma_start(wc_t[:], Wc_d.ap().rearrange("(c p) n -> p c n", p=128))

            for pt in range(PT):
                feat = persist.tile([128, EMB], F32, name=f"feat{pt}")
                nc.vector.tensor_tensor(feat[:, 0:512], ps_feat[pt][0][:],
                                        bx_b[:, 0:512], op=mybir.AluOpType.add)
                nc.vector.tensor_tensor(feat[:, 512:768], ps_feat[pt][1][:],
                                        bx_b[:, 512:768], op=mybir.AluOpType.add)
                nc.scalar.activation(feat[:], feat[:],
                                     mybir.ActivationFunctionType.Relu,
                                     bias=0.0, scale=1.0)

                stats = tmpp.tile([128, 3, 6], F32, name="stats")
                f_re = feat.rearrange("p (c f) -> p c f", c=3)
                for c in range(3):
                    nc.vector.bn_stats(stats[:, c, :], f_re[:, c, :])
                mv = tmpp.tile([128, 2], F32, name="mv")
                nc.vector.bn_aggr(mv[:], stats[:])
                sd = tmpp.tile([128, 1], F32, name="sd")
                nc.scalar.activation(sd[:], mv[:, 1:2],
                                     mybir.ActivationFunctionType.Sqrt,
                                     bias=eps_t[:], scale=1.0)
                rstd = tmpp.tile([128, 1], F32, name="rstd")
                nc.vector.reciprocal(rstd[:], sd[:])

                ln = persist.tile([128, EMB], F32, name=f"ln{pt}")
                nc.vector.tensor_scalar(ln[:], feat[:], mv[:, 0:1], rstd[:],
                                        op0=mybir.AluOpType.subtract,
                                        op1=mybir.AluOpType.mult)
                nc.vector.tensor_tensor(ln[:], ln[:], lng_b[:],
                                        op=mybir.AluOpType.mult)
                nc.vector.tensor_tensor(ln[:], ln[:], lnb_b[:],
                                        op=mybir.AluOpType.add)

                lnT = persist.tile([128, CT, 128], F32, name=f"lnT{pt}")
                for ct in range(CT):
                    ps_tr2 = psg.tile([128, 128], F32, name="gen")
                    nc.tensor.transpose(ps_tr2[:], ln[:, ct * 128:(ct + 1) * 128],
                                        ident[:])
                    nc.scalar.copy(lnT[:, ct, :], ps_tr2[:])

                ps_lg = psg.tile([128, NL], F32, name="gen")
                for ct in range(CT):
                    nc.tensor.matmul(ps_lg[:], lnT[:, ct, :], wc_t[:, ct, :],
                                     start=(ct == 0), stop=(ct == CT - 1))
                out_sb = tmpp.tile([128, NL], F32, name="out_sb")
                nc.scalar.copy(out_sb[:], ps_lg[:])
                nc.scalar.dma_start(out_d.ap()[pt * 128:(pt + 1) * 128, :], out_sb[:])

    nc.compile()
    return nc


_NC_CACHE = []


def _get_module():
    if not _NC_CACHE:
        _NC_CACHE.append(_build_module())
    return _NC_CACHE[0]


_EBC = np.zeros((2, 128), np.float16)
_EBC[0, :64] = 1.0
_EBC[1, 64:] = 1.0


def _build_inputs(seq, starts, ends, mention_mask, W_head, b_head, W_tail, b_tail,
                  W_ext, b_ext, ln_g, ln_b, W_cls):
    seq = np.asarray(seq, np.float32)
    starts = np.asarray(starts, np.int64)
    ends = np.asarray(ends, np.int64)
    mask = np.asarray(mention_mask, np.float32)

    # per-document entity selection matrix: ent = Sb^T @ seq[b]
    S_b = np.zeros((B, L, E), np.float32)
    denom = np.maximum(mask.sum(axis=2), 1.0)          # [B, E]
    w = mask * 0.5 / denom[:, :, None]                 # [B, E, M]
    for b in range(B):
        for e in range(E):
            np.add.at(S_b[b, :, e], starts[b, e] + 1, w[b, e])
            np.add.at(S_b[b, :, e], ends[b, e], w[b, e])

    cls_col = np.zeros((L, 1), np.float32)
    cls_col[0, 0] = 1.0

    shared = {
        "Wh": np.ascontiguousarray(np.asarray(W_head, np.float32).astype(np.float16)),
        "Wt": np.ascontiguousarray(np.asarray(W_tail, np.float32).astype(np.float16)),
        "bh": np.ascontiguousarray(np.asarray(b_head, np.float32).reshape(CT, 128).T),
        "bt": np.ascontiguousarray(np.asarray(b_tail, np.float32).reshape(CT, 128).T),
        "Wx": np.ascontiguousarray(np.asarray(W_ext).astype(np.float16)),
        "Ebc": _EBC,
        "bx": np.ascontiguousarray(np.broadcast_to(np.asarray(b_ext, np.float32), (128, EMB))),
        "lng": np.ascontiguousarray(np.broadcast_to(np.asarray(ln_g, np.float32), (128, EMB))),
        "lnb": np.ascontiguousarray(np.broadcast_to(np.asarray(ln_b, np.float32), (128, EMB))),
        "Wc": np.ascontiguousarray(W_cls, dtype=np.float32),
    }
    in_maps = []
    for core in range(N_CORES):
        b, ib = core // 4, core % 4
        S_core = np.concatenate(
            [S_b[b][:, ib * IB:(ib + 1) * IB], S_b[b], cls_col], axis=1)
        in_maps.append({
            "seq": np.ascontiguousarray(seq[b].astype(np.float16)),
            "S": np.ascontiguousarray(S_core.astype(np.float16)),
            **shared,
        })
    return in_maps


def kernel(**inputs) -> np.ndarray:
    nc = _get_module()
    in_maps = _build_inputs(**inputs)
    res = run_bass_kernel_spmd(nc, in_maps, core_ids=list(range(N_CORES)))
    outs = np.stack([res.results[c]["out"] for c in range(N_CORES)])  # [8,256,97]
    return outs.reshape(B, 4, IB, E, NL).reshape(B, E, E, NL)



# revision 29
# speedup vs baseline: 1.4781x; 1.0044x over previous
"""DocRE model kernel for 8 Trainium2 NeuronCores.

Data-parallel over the pair grid: core = b*4 + ib owns document b and
i-rows [8*ib, 8*ib+8) of the 32x32 entity-pair grid (256 pairs/core).
All weights are replicated; W_ext (49152x768) is streamed from HBM
through a float32r matmul with the group-bilinear feature tiles
materialized on-chip.
"""

import numpy as np

import concourse.bacc as bacc
import concourse.bass as bass
import concourse.tile as tile
from concourse import mybir
from concourse.bass_utils import run_bass_kernel_spmd
from concourse.masks import make_identity

F32 = mybir.dt.float32
F32R = mybir.dt.float32r
F16 = mybir.dt.float16

B, L, H = 2, 1024, 768
E, M = 32, 4
EMB, BLK, NL = 768, 64, 97
G = EMB // BLK  # 12
LN_EPS = 1e-12

N_CORES = 8
IB = E // (N_CORES // B)     # 8 i-rows per core
NPAIR = IB * E               # 256 pairs per core
PT = NPAIR // 128            # 2 pair-tiles
KT = EMB * BLK // 128        # 384 k-tiles
CT = EMB // 128              # 6 feature chunks
KC = H // 128                # 6 contraction chunks of H
LC = L // 128                # 8 chunks of L
NENT = IB + E + 1            # 41 cols: [my 8 entities | all 32 | cls]


def _build_module():
    nc = bacc.Bacc("TRN2", target_bir_lowering=False, debug=False)

    seq_d = nc.dram_tensor("seq", [L, H], F16, kind="ExternalInput")
    S_d = nc.dram_tensor("S", [L, NENT], F16, kind="ExternalInput")
    Wh_d = nc.dram_tensor("Wh", [3 * H, EMB], F16, kind="ExternalInput")
    Wt_d = nc.dram_tensor("Wt", [3 * H, EMB], F16, kind="ExternalInput")
    bh_d = nc.dram_tensor("bh", [128, CT], F32, kind="ExternalInput")
    bt_d = nc.dram_tensor("bt", [128, CT], F32, kind="ExternalInput")
    Wx_d = nc.dram_tensor("Wx", [EMB * BLK, EMB], F16, kind="ExternalInput")
    Ebc_d = nc.dram_tensor("Ebc", [2, 128], F16, kind="ExternalInput")
    bx_d = nc.dram_tensor("bx", [128, EMB], F32, kind="ExternalInput")
    lng_d = nc.dram_tensor("lng", [128, EMB], F32, kind="ExternalInput")
    lnb_d = nc.dram_tensor("lnb", [128, EMB], F32, kind="ExternalInput")
    Wc_d = nc.dram_tensor("Wc", [EMB, NL], F32, kind="ExternalInput")
    out_d = nc.dram_tensor("out", [NPAIR, NL], F32, kind="ExternalOutput")

    with tile.TileContext(nc) as tc:
        with (
            tc.tile_pool(name="persist", bufs=1) as persist,
            tc.tile_pool(name="seqp", bufs=1) as seqp,
            tc.tile_pool(name="whp", bufs=6) as whp,
            tc.tile_pool(name="wxp", bufs=10) as wxp,
            tc.tile_pool(name="blp", bufs=8) as blp,
            tc.tile_pool(name="hsgp", bufs=4) as hsgp,
            tc.tile_pool(name="tmpp", bufs=3) as tmpp,
            tc.tile_pool(name="dramp", bufs=1, space="DRAM") as dramp,
            tc.tile_pool(name="psf", bufs=1, space="PSUM") as psf,
            tc.tile_pool(name="psg", bufs=2, space="PSUM") as psg,
            tc.tile_pool(name="psb", bufs=2, space="PSUM") as psb,
        ):
            ident = persist.tile([128, 128], F32, name="ident")
            make_identity(nc, ident[:])

            # ---- per-column constants broadcast to all partitions ----
            # (allocated here; the 1.2MB of DMAs are issued after phase P so
            # they don't delay the phase E/A input streams at the head)
            bx_b = persist.tile([128, EMB], F32, name="bx_b")
            lng_b = persist.tile([128, EMB], F32, name="lng_b")
            lnb_b = persist.tile([128, EMB], F32, name="lnb_b")

            eps_t = persist.tile([128, 1], F32, name="eps")
            nc.vector.memset(eps_t[:], LN_EPS)

            # ---- phase E: entity pooling  ent = S^T @ seq ----
            # inputs first: S in one DMA, seq in two, so the first matmul's
            # inputs aren't stuck behind a queue of constant loads.
            seq_t = seqp.tile([128, LC, H], F16, name="seq_t")
            S_t = seqp.tile([128, LC, NENT], F16, name="S_t")
            seq_re = seq_d.ap().rearrange("(c p) h -> p c h", p=128)
            S_re = S_d.ap().rearrange("(c p) n -> p c n", p=128)
            nc.sync.dma_start(S_t[:], S_re)
            nc.sync.dma_start(seq_t[:, 0:LC // 2, :], seq_re[:, 0:LC // 2, :])
            nc.sync.dma_start(seq_t[:, LC // 2:LC, :], seq_re[:, LC // 2:LC, :])

            E_t = persist.tile([2, 128], F16, name="E_t")
            nc.sync.dma_start(E_t[:], Ebc_d.ap())
            # per-partition bias chunks bh/bt: [128, CT]
            bh_t = persist.tile([128, CT], F32, name="bh_t")
            bt_t = persist.tile([128, CT], F32, name="bt_t")
            for tile_, src in ((bh_t, bh_d), (bt_t, bt_d)):
                nc.sync.dma_start(tile_[:], src.ap())

            ps_e0 = psg.tile([NENT, 512], F32, name="gen")
            ps_e1 = psg.tile([NENT, 256], F32, name="gen")
            for kc in range(LC):
                nc.tensor.matmul(ps_e0[:], S_t[:, kc, :], seq_t[:, kc, 0:512],
                                 start=(kc == 0), stop=(kc == LC - 1))
                nc.tensor.matmul(ps_e1[:], S_t[:, kc, :], seq_t[:, kc, 512:768],
                                 start=(kc == 0), stop=(kc == LC - 1))
            ent_nat = persist.tile([NENT, H], F32, name="ent_nat")
            nc.scalar.copy(ent_nat[:, 0:512], ps_e0[:])
            nc.scalar.copy(ent_nat[:, 512:768], ps_e1[:])

            # transpose ent -> entT [h, NENT]  (f32r: feeds phase-A matmuls)
            entT = persist.tile([128, KC, NENT], F16, name="entT")
            for kc in range(KC):
                ps_tr = psg.tile([128, NENT], F32, name="gen")
                nc.tensor.transpose(ps_tr[:], ent_nat[:, kc * 128:(kc + 1) * 128],
                                    ident[:NENT, :NENT])
                nc.scalar.copy(entT[:, kc, :], ps_tr[:])

            # ---- phase A: A/B/C projections ----
            # natural layout first: X_nat = ent @ W_block  [41, 768], then
            # PE-transpose into ABCD[ct][:, m, :] ([c,41], m: Ah,Bh,At,Bt).
            ABCD = []
            for ct in range(CT):
                abcd_alloc = persist.tile([128, 4, NENT + 1], F32, name=f"abcd{ct}")
                nc.vector.memset(abcd_alloc[:], 0.0)
                ABCD.append(abcd_alloc)

            # feat accumulators allocated early: phase-A chains borrow these
            # idle PSUM banks so three chains can run concurrently.
            ps_feat = [[psf.tile([128, 512], F32, name=f"pf{pt}a"),
                        psf.tile([128, 256], F32, name=f"pf{pt}b")]
                       for pt in range(PT)]

            def emit_ab_chain(m, w_d, blk, ps_pair=None):
                if ps_pair is None:
                    ps_n0 = psg.tile([NENT, 512], F32, name="gen")
                    ps_n1 = psg.tile([NENT, 256], F32, name="gen")
                else:
                    ps_n0 = ps_pair[0][:NENT, :]
                    ps_n1 = ps_pair[1][:NENT, :]
                for kc2 in range(KC // 2):
                    w_t = whp.tile([128, 2, EMB], F16, name="w_t")
                    nc.scalar.dma_start(
                        w_t[:], w_d.ap()[blk * H + kc2 * 256: blk * H + (kc2 + 1) * 256, :]
                        .rearrange("(q p) e -> p q e", q=2))
                    for qq in range(2):
                        kc = 2 * kc2 + qq
                        nc.tensor.matmul(ps_n0[:], entT[:, kc, :], w_t[:, qq, 0:512],
                                         start=(kc == 0), stop=(kc == KC - 1))
                        nc.tensor.matmul(ps_n1[:], entT[:, kc, :], w_t[:, qq, 512:768],
                                         start=(kc == 0), stop=(kc == KC - 1))
                x_nat = tmpp.tile([NENT, EMB], F32, name="x_nat")
                nc.scalar.copy(x_nat[:, 0:512], ps_n0[:])
                nc.scalar.copy(x_nat[:, 512:768], ps_n1[:])
                for ct in range(CT):
                    ps_tr = psg.tile([128, NENT], F32, name="gen")
                    nc.tensor.transpose(ps_tr[:], x_nat[:, ct * 128:(ct + 1) * 128],
                                        ident[:NENT, :NENT])
                    nc.scalar.copy(ABCD[ct][:, m, 0:NENT], ps_tr[:])

            def emit_c_chain(m_sel, w_d, bias_t):
                ps_c0 = psg.tile([NENT, 512], F32, name="gen")
                ps_c1 = psg.tile([NENT, 256], F32, name="gen")
                for kc2 in range(KC // 2):
                    w_t = whp.tile([128, 2, EMB], F16, name="w_t")
                    nc.scalar.dma_start(
                        w_t[:], w_d.ap()[2 * H + kc2 * 256: 2 * H + (kc2 + 1) * 256, :]
                        .rearrange("(q p) e -> p q e", q=2))
                    for qq in range(2):
                        kc = 2 * kc2 + qq
                        nc.tensor.matmul(ps_c0[:1, :], entT[:, kc, IB + E:IB + E + 1],
                                         w_t[:, qq, 0:512],
                                         start=(kc == 0), stop=(kc == KC - 1))
                        nc.tensor.matmul(ps_c1[:1, :], entT[:, kc, IB + E:IB + E + 1],
                                         w_t[:, qq, 512:768],
                                         start=(kc == 0), stop=(kc == KC - 1))
                c_nat = tmpp.tile([1, EMB], F32, name="c_nat")
                nc.scalar.copy(c_nat[:, 0:512], ps_c0[:1, :])
                nc.scalar.copy(c_nat[:, 512:768], ps_c1[:1, :])
                for ct in range(CT):
                    ps_tr = psg.tile([128, NENT], F32, name="gen")
                    nc.tensor.transpose(ps_tr[:, 0:1],
                                        c_nat[:, ct * 128:(ct + 1) * 128],
                                        ident[:1, :1])
                    nc.vector.tensor_tensor(ABCD[ct][:, m_sel, NENT:NENT + 1],
                                            ps_tr[:, 0:1],
                                            bias_t[:, ct:ct + 1],
                                            op=mybir.AluOpType.add)

            emit_c_chain(3, Wt_d, bt_t)
            emit_ab_chain(2, Wt_d, 0, ps_feat[0])
            emit_ab_chain(3, Wt_d, 1, ps_feat[1])


            # ---- phase P ts-side: tsdup generated from duplicated ABCD ----
            # col = pt*128 + il*32 + j ; i = 8*ib + pt*4 + il
            hsT = persist.tile([128, CT, 2 * 128], F16, name="hsT")
            tsdup = persist.tile([128, G, 2 * 128], F16, name="tsdup")
            hs_dram = dramp.tile([EMB, 2 * 128], F16, name="hs_dram")
            NE2 = NENT + 1

            def colview(tile_, m, col0, ap_pat):
                return bass.AP(tensor=tile_.tensor,
                               offset=tile_.offset + m * NE2 + col0,
                               ap=[tile_.ap[0]] + ap_pat)

            for ct in range(CT):
                abcd_t = ABCD[ct]
                for half in range(2):
                    g = 2 * ct + half
                    dup_t = tmpp.tile([128, 4, NE2], F32, name="dup")
                    src_ab = abcd_t[half * 64:half * 64 + 64, :, :]
                    nc.scalar.dma_start(dup_t[0:64, :, :], src_ab)
                    nc.scalar.dma_start(dup_t[64:128, :, :], src_ab)
                    tmp2 = tmpp.tile([128, 8, 32], F32, name="tmp")
                    nc.vector.tensor_tensor(
                        tmp2[:], colview(dup_t, 2, IB, [[0, 8], [1, 32]]),
                        colview(dup_t, 3, 0, [[1, 8], [0, 32]]),
                        op=mybir.AluOpType.add)
                    nc.scalar.activation(
                        tsdup[:, g, :].rearrange("p (a b) -> p a b", a=8),
                        tmp2[:], mybir.ActivationFunctionType.Tanh,
                        bias=dup_t[:, 3, NENT:NENT + 1], scale=1.0)

            # ---- head-side projections, then hs generation ----
            emit_c_chain(0, Wh_d, bh_t)
            emit_ab_chain(0, Wh_d, 0, ps_feat[0])
            emit_ab_chain(1, Wh_d, 1, ps_feat[1])
            for ct in range(CT):
                abcd_t = ABCD[ct]
                tmp = tmpp.tile([128, 8, 32], F32, name="tmp")
                nc.vector.tensor_tensor(
                    tmp[:], colview(abcd_t, 0, 0, [[1, 8], [0, 32]]),
                    colview(abcd_t, 1, IB, [[0, 8], [1, 32]]),
                    op=mybir.AluOpType.add)
                nc.scalar.activation(
                    hsT[:, ct, :].rearrange("p (a b) -> p a b", a=8),
                    tmp[:], mybir.ActivationFunctionType.Tanh,
                    bias=abcd_t[:, 0, NENT:NENT + 1], scale=1.0)
                nc.scalar.dma_start(hs_dram[ct * 128:(ct + 1) * 128, :],
                                    hsT[:, ct, :])

            # phase-L constants: issued now so they queue behind the
            # phase E/A input streams but ahead of the bulk of W_ext.
            for tile_, src in ((bx_b, bx_d), (lng_b, lng_d), (lnb_b, lnb_d)):
                nc.sync.dma_start(tile_[:], src.ap())

            # ---- phase M: main contraction over W_ext ----
            # software pipeline: broadcasts for upcoming k-tiles issue before
            # the current k-tiles' main matmuls so the DVE multiply latency is
            # hidden behind PE work.  Broadcast matmuls are emitted two at a
            # time so they share one E_t LDWEIGHTS; main matmuls stream
            # 256-col chunks (measured faster per column than 512-col).
            hsg_tiles = {}

            def stage_group(g):
                # stage group g's 64 hs rows into partitions 0-1, pair-major:
                # hsg[r, tq, 256*q + p] = hs row (g*64 + 4*tq + 2*q + r)
                hsg_tiles[g] = hsgp.tile([2, 16, 512], F16, name="hsg")
                nc.scalar.dma_start(
                    hsg_tiles[g][:].rearrange("r tq (q p) -> r tq q p", q=2),
                    bass.AP(tensor=hs_dram.tensor,
                            offset=hs_dram.offset + g * 64 * 2 * 128,
                            ap=[[256, 2], [4 * 256, 16], [2 * 256, 2], [1, 256]]))

            def emit_bc_quad(ktq):
                # two [2,512] rhs matmuls sharing one E_t weight load;
                # covers k-tiles 4*ktq .. 4*ktq+3
                kt0 = 4 * ktq
                g = kt0 // 32
                tq = (kt0 % 32) // 2
                if kt0 % 32 == 24 and g + 1 < G:
                    stage_group(g + 1)
                bc_a = psb.tile([128, 512], F32, name="bc_ps")
                bc_b = psb.tile([128, 512], F32, name="bc_ps")
                nc.tensor.matmul(bc_a[:], E_t[:], hsg_tiles[g][:, tq, :],
                                 start=True, stop=True)
                nc.tensor.matmul(bc_b[:], E_t[:], hsg_tiles[g][:, tq + 1, :],
                                 start=True, stop=True)
                return bc_a, bc_b

            stage_group(0)
            bc_cur = emit_bc_quad(0)
            for kt2 in range(KT // 2):
                wx_t = wxp.tile([128, 2, EMB], F16, name="wx_t")
                nc.sync.dma_start(
                    wx_t[:],
                    Wx_d.ap()[kt2 * 256:(kt2 + 1) * 256, :]
                    .rearrange("(q p) e -> p q e", q=2))
                for q in range(2):
                    kt = 2 * kt2 + q
                    g = kt // 32
                    bc_ps = bc_cur[kt2 % 2]
                    bl_t = blp.tile([128, 2 * 128], F16, name="bl_t")
                    nc.vector.tensor_tensor(bl_t[:],
                                            bc_ps[:, q * 256:(q + 1) * 256],
                                            tsdup[:, g, :],
                                            op=mybir.AluOpType.mult)
                    if kt % 4 == 3 and kt + 1 < KT:
                        bc_cur = emit_bc_quad((kt + 1) // 4)
                    first, last = (kt == 0), (kt == KT - 1)
                    for pt in range(PT):
                        lhsT = bl_t[:, pt * 128:(pt + 1) * 128]
                        if first or last:
                            nc.tensor.matmul(ps_feat[pt][0][:], lhsT,
                                             wx_t[:, q, 0:512],
                                             start=first, stop=last)
                        else:
                            nc.tensor.matmul(ps_feat[pt][0][:, 0:256], lhsT,
                                             wx_t[:, q, 0:256],
                                             start=False, stop=False)
                            nc.tensor.matmul(ps_feat[pt][0][:, 256:512], lhsT,
                                             wx_t[:, q, 256:512],
                                             start=False, stop=False)
                        nc.tensor.matmul(ps_feat[pt][1][:], lhsT,
                                         wx_t[:, q, 512:768],
                                         start=first, stop=last)

            # ---- phase L: bias, relu, layernorm, classifier ----
            wc_t = persist.tile([128, CT, NL], F32, name="wc_t")
            nc.sync.dma_start(wc_t[:], Wc_d.ap().rearrange("(c p) n -> p c n", p=128))

            for pt in range(PT):
                feat = persist.tile([128, EMB], F32, name=f"feat{pt}")
                nc.vector.tensor_tensor(feat[:, 0:512], ps_feat[pt][0][:],
                                        bx_b[:, 0:512], op=mybir.AluOpType.add)
                nc.vector.tensor_tensor(feat[:, 512:768], ps_feat[pt][1][:],
                                        bx_b[:, 512:768], op=mybir.AluOpType.add)
                nc.scalar.activation(feat[:], feat[:],
                                     mybir.ActivationFunctionType.Relu,
                                     bias=0.0, scale=1.0)

                stats = tmpp.tile([128, 3, 6], F32, name="stats")
                f_re = feat.rearrange("p (c f) -> p c f", c=3)
                for c in range(3):
                    nc.vector.bn_stats(stats[:, c, :], f_re[:, c, :])
                mv = tmpp.tile([128, 2], F32, name="mv")
                nc.vector.bn_aggr(mv[:], stats[:])
                sd = tmpp.tile([128, 1], F32, name="sd")
                nc.scalar.activation(sd[:], mv[:, 1:2],
                                     mybir.ActivationFunctionType.Sqrt,
                                     bias=eps_t[:], scale=1.0)
                rstd = tmpp.tile([128, 1], F32, name="rstd")
                nc.vector.reciprocal(rstd[:], sd[:])

                ln = persist.tile([128, EMB], F32, name=f"ln{pt}")
                nc.vector.tensor_scalar(ln[:], feat[:], mv[:, 0:1], rstd[:],
                                        op0=mybir.AluOpType.subtract,
                                        op1=mybir.AluOpType.mult)
                nc.vector.tensor_tensor(ln[:], ln[:], lng_b[:],
                                        op=mybir.AluOpType.mult)
                nc.vector.tensor_tensor(ln[:], ln[:], lnb_b[:],
                                        op=mybir.AluOpType.add)

                lnT = persist.tile([128, CT, 128], F32, name=f"lnT{pt}")
                for ct in range(CT):
                    ps_tr2 = psg.tile([128, 128], F32, name="gen")
                    nc.tensor.transpose(ps_tr2[:], ln[:, ct * 128:(ct + 1) * 128],
                                        ident[:])
                    nc.scalar.copy(lnT[:, ct, :], ps_tr2[:])

                ps_lg = psg.tile([128, NL], F32, name="gen")
                for ct in range(CT):
                    nc.tensor.matmul(ps_lg[:], lnT[:, ct, :], wc_t[:, ct, :],
                                     start=(ct == 0), stop=(ct == CT - 1))
                out_sb = tmpp.tile([128, NL], F32, name="out_sb")
                nc.scalar.copy(out_sb[:], ps_lg[:])
                nc.scalar.dma_start(out_d.ap()[pt * 128:(pt + 1) * 128, :], out_sb[:])

    nc.compile()
    return nc


_NC_CACHE = []


def _get_module():
    if not _NC_CACHE:
        _NC_CACHE.append(_build_module())
    return _NC_CACHE[0]


_EBC = np.zeros((2, 128), np.float16)
_EBC[0, :64] = 1.0
_EBC[1, 64:] = 1.0


def _build_inputs(seq, starts, ends, mention_mask, W_head, b_head, W_tail, b_tail,
                  W_ext, b_ext, ln_g, ln_b, W_cls):
    seq = np.asarray(seq, np.float32)
    starts = np.asarray(starts, np.int64)
    ends = np.asarray(ends, np.int64)
    mask = np.asarray(mention_mask, np.float32)

    # per-document entity selection matrix: ent = Sb^T @ seq[b]
    S_b = np.zeros((B, L, E), np.float32)
    denom = np.maximum(mask.sum(axis=2), 1.0)          # [B, E]
    w = mask * 0.5 / denom[:, :, None]                 # [B, E, M]
    for b in range(B):
        for e in range(E):
            np.add.at(S_b[b, :, e], starts[b, e] + 1, w[b, e])
            np.add.at(S_b[b, :, e], ends[b, e], w[b, e])

    cls_col = np.zeros((L, 1), np.float32)
    cls_col[0, 0] = 1.0

    shared = {
        "Wh": np.ascontiguousarray(np.asarray(W_head, np.float32).astype(np.float16)),
        "Wt": np.ascontiguousarray(np.asarray(W_tail, np.float32).astype(np.float16)),
        "bh": np.ascontiguousarray(np.asarray(b_head, np.float32).reshape(CT, 128).T),
        "bt": np.ascontiguousarray(np.asarray(b_tail, np.float32).reshape(CT, 128).T),
        "Wx": np.ascontiguousarray(np.asarray(W_ext).astype(np.float16)),
        "Ebc": _EBC,
        "bx": np.ascontiguousarray(np.broadcast_to(np.asarray(b_ext, np.float32), (128, EMB))),
        "lng": np.ascontiguousarray(np.broadcast_to(np.asarray(ln_g, np.float32), (128, EMB))),
        "lnb": np.ascontiguousarray(np.broadcast_to(np.asarray(ln_b, np.float32), (128, EMB))),
        "Wc": np.ascontiguousarray(W_cls, dtype=np.float32),
    }
    in_maps = []
    for core in range(N_CORES):
        b, ib = core // 4, core % 4
        S_core = np.concatenate(
            [S_b[b][:, ib * IB:(ib + 1) * IB], S_b[b], cls_col], axis=1)
        in_maps.append({
            "seq": np.ascontiguousarray(seq[b].astype(np.float16)),
            "S": np.ascontiguousarray(S_core.astype(np.float16)),
            **shared,
        })
    return in_maps


def kernel(**inputs) -> np.ndarray:
    nc = _get_module()
    in_maps = _build_inputs(**inputs)
    res = run_bass_kernel_spmd(nc, in_maps, core_ids=list(range(N_CORES)))
    outs = np.stack([res.results[c]["out"] for c in range(N_CORES)])  # [8,256,97]
    return outs.reshape(B, 4, IB, E, NL).reshape(B, E, E, NL)



# revision 30
# speedup vs baseline: 1.4894x; 1.0077x over previous
"""DocRE model kernel for 8 Trainium2 NeuronCores.

Data-parallel over the pair grid: core = b*4 + ib owns document b and
i-rows [8*ib, 8*ib+8) of the 32x32 entity-pair grid (256 pairs/core).
All weights are replicated; W_ext (49152x768) is streamed from HBM
through a float32r matmul with the group-bilinear feature tiles
materialized on-chip.
"""

import numpy as np

import concourse.bacc as bacc
import concourse.bass as bass
import concourse.tile as tile
from concourse import mybir
from concourse.bass_utils import run_bass_kernel_spmd
from concourse.masks import make_identity

F32 = mybir.dt.float32
F32R = mybir.dt.float32r
F16 = mybir.dt.float16

B, L, H = 2, 1024, 768
E, M = 32, 4
EMB, BLK, NL = 768, 64, 97
G = EMB // BLK  # 12
LN_EPS = 1e-12

N_CORES = 8
IB = E // (N_CORES // B)     # 8 i-rows per core
NPAIR = IB * E               # 256 pairs per core
PT = NPAIR // 128            # 2 pair-tiles
KT = EMB * BLK // 128        # 384 k-tiles
CT = EMB // 128              # 6 feature chunks
KC = H // 128                # 6 contraction chunks of H
LC = L // 128                # 8 chunks of L
NENT = IB + E + 1            # 41 cols: [my 8 entities | all 32 | cls]


def _build_module():
    nc = bacc.Bacc("TRN2", target_bir_lowering=False, debug=False)

    seq_d = nc.dram_tensor("seq", [L, H], F16, kind="ExternalInput")
    S_d = nc.dram_tensor("S", [L, NENT], F16, kind="ExternalInput")
    Wh_d = nc.dram_tensor("Wh", [3 * H, EMB], F16, kind="ExternalInput")
    Wt_d = nc.dram_tensor("Wt", [3 * H, EMB], F16, kind="ExternalInput")
    bh_d = nc.dram_tensor("bh", [128, CT], F32, kind="ExternalInput")
    bt_d = nc.dram_tensor("bt", [128, CT], F32, kind="ExternalInput")
    Wx_d = nc.dram_tensor("Wx", [EMB * BLK, EMB], F16, kind="ExternalInput")
    Ebc_d = nc.dram_tensor("Ebc", [2, 128], F16, kind="ExternalInput")
    bx_d = nc.dram_tensor("bx", [128, EMB], F32, kind="ExternalInput")
    lng_d = nc.dram_tensor("lng", [128, EMB], F32, kind="ExternalInput")
    lnb_d = nc.dram_tensor("lnb", [128, EMB], F32, kind="ExternalInput")
    Wc_d = nc.dram_tensor("Wc", [EMB, NL], F32, kind="ExternalInput")
    out_d = nc.dram_tensor("out", [NPAIR, NL], F32, kind="ExternalOutput")

    with tile.TileContext(nc) as tc:
        with (
            tc.tile_pool(name="persist", bufs=1) as persist,
            tc.tile_pool(name="seqp", bufs=1) as seqp,
            tc.tile_pool(name="whp", bufs=6) as whp,
            tc.tile_pool(name="wxp", bufs=10) as wxp,
            tc.tile_pool(name="blp", bufs=8) as blp,
            tc.tile_pool(name="hsgp", bufs=4) as hsgp,
            tc.tile_pool(name="tmpp", bufs=3) as tmpp,
            tc.tile_pool(name="dramp", bufs=1, space="DRAM") as dramp,
            tc.tile_pool(name="psf", bufs=1, space="PSUM") as psf,
            tc.tile_pool(name="psg", bufs=2, space="PSUM") as psg,
            tc.tile_pool(name="psb", bufs=2, space="PSUM") as psb,
        ):
            ident = persist.tile([128, 128], F32, name="ident")
            make_identity(nc, ident[:])

            # ---- per-column constants broadcast to all partitions ----
            # (allocated here; the 1.2MB of DMAs are issued after phase P so
            # they don't delay the phase E/A input streams at the head)
            bx_b = persist.tile([128, EMB], F32, name="bx_b")
            lng_b = persist.tile([128, EMB], F32, name="lng_b")
            lnb_b = persist.tile([128, EMB], F32, name="lnb_b")

            eps_t = persist.tile([128, 1], F32, name="eps")
            nc.vector.memset(eps_t[:], LN_EPS)

            # ---- phase E: entity pooling  ent = S^T @ seq ----
            # inputs first: S in one DMA, seq in two, so the first matmul's
            # inputs aren't stuck behind a queue of constant loads.
            seq_t = seqp.tile([128, LC, H], F16, name="seq_t")
            S_t = seqp.tile([128, LC, NENT], F16, name="S_t")
            seq_re = seq_d.ap().rearrange("(c p) h -> p c h", p=128)
            S_re = S_d.ap().rearrange("(c p) n -> p c n", p=128)
            nc.sync.dma_start(S_t[:], S_re)
            nc.sync.dma_start(seq_t[:, 0:LC // 2, :], seq_re[:, 0:LC // 2, :])
            nc.sync.dma_start(seq_t[:, LC // 2:LC, :], seq_re[:, LC // 2:LC, :])

            E_t = persist.tile([2, 128], F16, name="E_t")
            nc.sync.dma_start(E_t[:], Ebc_d.ap())
            # per-partition bias chunks bh/bt: [128, CT]
            bh_t = persist.tile([128, CT], F32, name="bh_t")
            bt_t = persist.tile([128, CT], F32, name="bt_t")
            for tile_, src in ((bh_t, bh_d), (bt_t, bt_d)):
                nc.sync.dma_start(tile_[:], src.ap())

            ps_e0 = psg.tile([NENT, 512], F32, name="gen")
            ps_e1 = psg.tile([NENT, 256], F32, name="gen")
            for kc in range(LC):
                nc.tensor.matmul(ps_e0[:], S_t[:, kc, :], seq_t[:, kc, 0:512],
                                 start=(kc == 0), stop=(kc == LC - 1))
                nc.tensor.matmul(ps_e1[:], S_t[:, kc, :], seq_t[:, kc, 512:768],
                                 start=(kc == 0), stop=(kc == LC - 1))
            ent_nat = persist.tile([NENT, H], F32, name="ent_nat")
            nc.scalar.copy(ent_nat[:, 0:512], ps_e0[:])
            nc.scalar.copy(ent_nat[:, 512:768], ps_e1[:])

            # transpose ent -> entT [h, NENT]  (f32r: feeds phase-A matmuls)
            entT = persist.tile([128, KC, NENT], F16, name="entT")
            for kc in range(KC):
                ps_tr = psg.tile([128, NENT], F32, name="gen")
                nc.tensor.transpose(ps_tr[:], ent_nat[:, kc * 128:(kc + 1) * 128],
                                    ident[:NENT, :NENT])
                nc.scalar.copy(entT[:, kc, :], ps_tr[:])

            # ---- phase A: A/B/C projections, software-pipelined ----
            # natural layout first: X_nat = ent @ W_block  [41, 768], then
            # PE-transpose into ABCD[ct][:, m, :] ([c,41], m: Ah,Bh,At,Bt).
            # Four PSUM accumulator pairs (psg + both ps_feat + psb) keep up
            # to four chains in flight; each chain's post-processing is
            # emitted after the NEXT chain's matmuls so the PE stays busy
            # (and HAM-warm) while the chain weight DMAs stream in.
            ABCD = []
            for ct in range(CT):
                abcd_alloc = persist.tile([128, 4, NENT + 1], F32, name=f"abcd{ct}")
                nc.vector.memset(abcd_alloc[:], 0.0)
                ABCD.append(abcd_alloc)

            ps_feat = [[psf.tile([128, 512], F32, name=f"pf{pt}a"),
                        psf.tile([128, 256], F32, name=f"pf{pt}b")]
                       for pt in range(PT)]

            hsT = persist.tile([128, CT, 2 * 128], F16, name="hsT")
            tsdup = persist.tile([128, G, 2 * 128], F16, name="tsdup")
            hs_dram = dramp.tile([EMB, 2 * 128], F16, name="hs_dram")
            NE2 = NENT + 1

            def colview(tile_, m, col0, ap_pat):
                return bass.AP(tensor=tile_.tensor,
                               offset=tile_.offset + m * NE2 + col0,
                               ap=[tile_.ap[0]] + ap_pat)

            def chain_mms(w_d, blk, pair, cls_only=False):
                ps_n0, ps_n1 = pair
                c0, c1 = (IB + E, IB + E + 1) if cls_only else (0, NENT)
                mrows = 1 if cls_only else NENT
                for kc2 in range(KC // 2):
                    w_t = whp.tile([128, 2, EMB], F16, name="w_t")
                    nc.sync.dma_start(
                        w_t[:],
                        w_d.ap()[blk * H + kc2 * 256: blk * H + (kc2 + 1) * 256, :]
                        .rearrange("(q p) e -> p q e", q=2))
                    for qq in range(2):
                        kc = 2 * kc2 + qq
                        nc.tensor.matmul(ps_n0[:mrows, 0:512],
                                         entT[:, kc, c0:c1], w_t[:, qq, 0:512],
                                         start=(kc == 0), stop=(kc == KC - 1))
                        nc.tensor.matmul(ps_n1[:mrows, 0:256],
                                         entT[:, kc, c0:c1], w_t[:, qq, 512:768],
                                         start=(kc == 0), stop=(kc == KC - 1))

            def ab_post(m, pair):
                ps_n0, ps_n1 = pair
                x_nat = tmpp.tile([NENT, EMB], F32, name="x_nat")
                nc.scalar.copy(x_nat[:, 0:512], ps_n0[:NENT, 0:512])
                nc.scalar.copy(x_nat[:, 512:768], ps_n1[:NENT, 0:256])
                for ct in range(CT):
                    ps_tr = (ps_n0 if ct % 2 == 0 else ps_n1)[:, 0:NENT]
                    nc.tensor.transpose(ps_tr, x_nat[:, ct * 128:(ct + 1) * 128],
                                        ident[:NENT, :NENT])
                    nc.scalar.copy(ABCD[ct][:, m, 0:NENT], ps_tr)

            def c_post(m_sel, pair, bias_t):
                ps_c0, ps_c1 = pair
                c_nat = tmpp.tile([1, EMB], F32, name="c_nat")
                nc.scalar.copy(c_nat[:, 0:512], ps_c0[:1, 0:512])
                nc.scalar.copy(c_nat[:, 512:768], ps_c1[:1, 0:256])
                for ct in range(CT):
                    ps_tr = (ps_c0 if ct % 2 == 0 else ps_c1)[:, 0:1]
                    nc.tensor.transpose(ps_tr,
                                        c_nat[:, ct * 128:(ct + 1) * 128],
                                        ident[:1, :1])
                    nc.vector.tensor_tensor(ABCD[ct][:, m_sel, NENT:NENT + 1],
                                            ps_tr,
                                            bias_t[:, ct:ct + 1],
                                            op=mybir.AluOpType.add)

            def ts_post(ct):
                abcd_t = ABCD[ct]
                for half in range(2):
                    g = 2 * ct + half
                    dup_t = tmpp.tile([128, 4, NE2], F32, name="dup")
                    src_ab = abcd_t[half * 64:half * 64 + 64, :, :]
                    nc.scalar.dma_start(dup_t[0:64, :, :], src_ab)
                    nc.scalar.dma_start(dup_t[64:128, :, :], src_ab)
                    tmp2 = tmpp.tile([128, 8, 32], F32, name="tmp")
                    nc.vector.tensor_tensor(
                        tmp2[:], colview(dup_t, 2, IB, [[0, 8], [1, 32]]),
                        colview(dup_t, 3, 0, [[1, 8], [0, 32]]),
                        op=mybir.AluOpType.add)
                    nc.scalar.activation(
                        tsdup[:, g, :].rearrange("p (a b) -> p a b", a=8),
                        tmp2[:], mybir.ActivationFunctionType.Tanh,
                        bias=dup_t[:, 3, NENT:NENT + 1], scale=1.0)

            def hs_post(ct):
                abcd_t = ABCD[ct]
                tmp = tmpp.tile([128, 8, 32], F32, name="tmp")
                nc.vector.tensor_tensor(
                    tmp[:], colview(abcd_t, 0, 0, [[1, 8], [0, 32]]),
                    colview(abcd_t, 1, IB, [[0, 8], [1, 32]]),
                    op=mybir.AluOpType.add)
                nc.scalar.activation(
                    hsT[:, ct, :].rearrange("p (a b) -> p a b", a=8),
                    tmp[:], mybir.ActivationFunctionType.Tanh,
                    bias=abcd_t[:, 0, NENT:NENT + 1], scale=1.0)
                nc.scalar.dma_start(hs_dram[ct * 128:(ct + 1) * 128, :],
                                    hsT[:, ct, :])

            pairA = (psg.tile([128, 512], F32, name="gen"),
                     psg.tile([128, 256], F32, name="gen"))
            pairD = (psb.tile([128, 512], F32, name="bc_ps"),
                     psb.tile([128, 512], F32, name="bc_ps"))

            chain_mms(Wt_d, 2, pairA, cls_only=True)    # c_t
            chain_mms(Wt_d, 0, ps_feat[0])              # At
            c_post(3, pairA, bt_t)
            chain_mms(Wt_d, 1, ps_feat[1])              # Bt
            ab_post(2, ps_feat[0])
            chain_mms(Wh_d, 2, pairD, cls_only=True)    # c_h
            ab_post(3, ps_feat[1])
            for ct in range(CT):
                ts_post(ct)
            chain_mms(Wh_d, 0, ps_feat[0])              # Ah
            c_post(0, pairD, bh_t)
            chain_mms(Wh_d, 1, ps_feat[1])              # Bh
            ab_post(0, ps_feat[0])
            ab_post(1, ps_feat[1])
            for ct in range(CT):
                hs_post(ct)

            # phase-L constants: issued now so they queue behind the
            # phase E/A input streams but ahead of the bulk of W_ext.
            for tile_, src in ((bx_b, bx_d), (lng_b, lng_d), (lnb_b, lnb_d)):
                nc.sync.dma_start(tile_[:], src.ap())

            # ---- phase M: main contraction over W_ext ----
            # software pipeline: broadcasts for upcoming k-tiles issue before
            # the current k-tiles' main matmuls so the DVE multiply latency is
            # hidden behind PE work.  Broadcast matmuls are emitted two at a
            # time so they share one E_t LDWEIGHTS; main matmuls stream
            # 256-col chunks (measured faster per column than 512-col).
            hsg_tiles = {}

            def stage_group(g):
                # stage group g's 64 hs rows into partitions 0-1, pair-major:
                # hsg[r, tq, 256*q + p] = hs row (g*64 + 4*tq + 2*q + r)
                hsg_tiles[g] = hsgp.tile([2, 16, 512], F16, name="hsg")
                nc.scalar.dma_start(
                    hsg_tiles[g][:].rearrange("r tq (q p) -> r tq q p", q=2),
                    bass.AP(tensor=hs_dram.tensor,
                            offset=hs_dram.offset + g * 64 * 2 * 128,
                            ap=[[256, 2], [4 * 256, 16], [2 * 256, 2], [1, 256]]))

            def emit_bc_quad(ktq):
                # two [2,512] rhs matmuls sharing one E_t weight load;
                # covers k-tiles 4*ktq .. 4*ktq+3
                kt0 = 4 * ktq
                g = kt0 // 32
                tq = (kt0 % 32) // 2
                if kt0 % 32 == 24 and g + 1 < G:
                    stage_group(g + 1)
                bc_a = psb.tile([128, 512], F32, name="bc_ps")
                bc_b = psb.tile([128, 512], F32, name="bc_ps")
                nc.tensor.matmul(bc_a[:], E_t[:], hsg_tiles[g][:, tq, :],
                                 start=True, stop=True)
                nc.tensor.matmul(bc_b[:], E_t[:], hsg_tiles[g][:, tq + 1, :],
                                 start=True, stop=True)
                return bc_a, bc_b

            stage_group(0)
            bc_cur = emit_bc_quad(0)
            for kt2 in range(KT // 2):
                wx_t = wxp.tile([128, 2, EMB], F16, name="wx_t")
                nc.sync.dma_start(
                    wx_t[:],
                    Wx_d.ap()[kt2 * 256:(kt2 + 1) * 256, :]
                    .rearrange("(q p) e -> p q e", q=2))
                for q in range(2):
                    kt = 2 * kt2 + q
                    g = kt // 32
                    bc_ps = bc_cur[kt2 % 2]
                    bl_t = blp.tile([128, 2 * 128], F16, name="bl_t")
                    nc.vector.tensor_tensor(bl_t[:],
                                            bc_ps[:, q * 256:(q + 1) * 256],
                                            tsdup[:, g, :],
                                            op=mybir.AluOpType.mult)
                    if kt % 4 == 3 and kt + 1 < KT:
                        bc_cur = emit_bc_quad((kt + 1) // 4)
                    first, last = (kt == 0), (kt == KT - 1)
                    for pt in range(PT):
                        lhsT = bl_t[:, pt * 128:(pt + 1) * 128]
                        if first or last:
                            nc.tensor.matmul(ps_feat[pt][0][:], lhsT,
                                             wx_t[:, q, 0:512],
                                             start=first, stop=last)
                        else:
                            nc.tensor.matmul(ps_feat[pt][0][:, 0:256], lhsT,
                                             wx_t[:, q, 0:256],
                                             start=False, stop=False)
                            nc.tensor.matmul(ps_feat[pt][0][:, 256:512], lhsT,
                                             wx_t[:, q, 256:512],
                                             start=False, stop=False)
                        nc.tensor.matmul(ps_feat[pt][1][:], lhsT,
                                         wx_t[:, q, 512:768],
                                         start=first, stop=last)

            # ---- phase L: bias, relu, layernorm, classifier ----
            wc_t = persist.tile([128, CT, NL], F32, name="wc_t")
            nc.sync.dma_start(wc_t[:], Wc_d.ap().rearrange("(c p) n -> p c n", p=128))

            for pt in range(PT):
                feat = persist.tile([128, EMB], F32, name=f"feat{pt}")
                nc.vector.tensor_tensor(feat[:, 0:512], ps_feat[pt][0][:],
                                        bx_b[:, 0:512], op=mybir.AluOpType.add)
                nc.vector.tensor_tensor(feat[:, 512:768], ps_feat[pt][1][:],
                                        bx_b[:, 512:768], op=mybir.AluOpType.add)
                nc.scalar.activation(feat[:], feat[:],
                                     mybir.ActivationFunctionType.Relu,
                                     bias=0.0, scale=1.0)

                stats = tmpp.tile([128, 3, 6], F32, name="stats")
                f_re = feat.rearrange("p (c f) -> p c f", c=3)
                for c in range(3):
                    nc.vector.bn_stats(stats[:, c, :], f_re[:, c, :])
                mv = tmpp.tile([128, 2], F32, name="mv")
                nc.vector.bn_aggr(mv[:], stats[:])
                sd = tmpp.tile([128, 1], F32, name="sd")
                nc.scalar.activation(sd[:], mv[:, 1:2],
                                     mybir.ActivationFunctionType.Sqrt,
                                     bias=eps_t[:], scale=1.0)
                rstd = tmpp.tile([128, 1], F32, name="rstd")
                nc.vector.reciprocal(rstd[:], sd[:])

                ln = persist.tile([128, EMB], F32, name=f"ln{pt}")
                nc.vector.tensor_scalar(ln[:], feat[:], mv[:, 0:1], rstd[:],
                                        op0=mybir.AluOpType.subtract,
                                        op1=mybir.AluOpType.mult)
                nc.vector.tensor_tensor(ln[:], ln[:], lng_b[:],
                                        op=mybir.AluOpType.mult)
                nc.vector.tensor_tensor(ln[:], ln[:], lnb_b[:],
                                        op=mybir.AluOpType.add)

                lnT = persist.tile([128, CT, 128], F32, name=f"lnT{pt}")
                for ct in range(CT):
                    ps_tr2 = psg.tile([128, 128], F32, name="gen")
                    nc.tensor.transpose(ps_tr2[:], ln[:, ct * 128:(ct + 1) * 128],
                                        ident[:])
                    nc.scalar.copy(lnT[:, ct, :], ps_tr2[:])

                ps_lg = psg.tile([128, NL], F32, name="gen")
                for ct in range(CT):
                    nc.tensor.matmul(ps_lg[:], lnT[:, ct, :], wc_t[:, ct, :],
                                     start=(ct == 0), stop=(ct == CT - 1))
                out_sb = tmpp.tile([128, NL], F32, name="out_sb")
                nc.scalar.copy(out_sb[:], ps_lg[:])
                nc.scalar.dma_start(out_d.ap()[pt * 128:(pt + 1) * 128, :], out_sb[:])

    nc.compile()
    return nc


_NC_CACHE = []


def _get_module():
    if not _NC_CACHE:
        _NC_CACHE.append(_build_module())
    return _NC_CACHE[0]


_EBC = np.zeros((2, 128), np.float16)
_EBC[0, :64] = 1.0
_EBC[1, 64:] = 1.0


def _build_inputs(seq, starts, ends, mention_mask, W_head, b_head, W_tail, b_tail,
                  W_ext, b_ext, ln_g, ln_b, W_cls):
    seq = np.asarray(seq, np.float32)
    starts = np.asarray(starts, np.int64)
    ends = np.asarray(ends, np.int64)
    mask = np.asarray(mention_mask, np.float32)

    # per-document entity selection matrix: ent = Sb^T @ seq[b]
    S_b = np.zeros((B, L, E), np.float32)
    denom = np.maximum(mask.sum(axis=2), 1.0)          # [B, E]
    w = mask * 0.5 / denom[:, :, None]                 # [B, E, M]
    for b in range(B):
        for e in range(E):
            np.add.at(S_b[b, :, e], starts[b, e] + 1, w[b, e])
            np.add.at(S_b[b, :, e], ends[b, e], w[b, e])

    cls_col = np.zeros((L, 1), np.float32)
    cls_col[0, 0] = 1.0

    shared = {
        "Wh": np.ascontiguousarray(np.asarray(W_head, np.float32).astype(np.float16)),
        "Wt": np.ascontiguousarray(np.asarray(W_tail, np.float32).astype(np.float16)),
        "bh": np.ascontiguousarray(np.asarray(b_head, np.float32).reshape(CT, 128).T),
        "bt": np.ascontiguousarray(np.asarray(b_tail, np.float32).reshape(CT, 128).T),
        "Wx": np.ascontiguousarray(np.asarray(W_ext).astype(np.float16)),
        "Ebc": _EBC,
        "bx": np.ascontiguousarray(np.broadcast_to(np.asarray(b_ext, np.float32), (128, EMB))),
        "lng": np.ascontiguousarray(np.broadcast_to(np.asarray(ln_g, np.float32), (128, EMB))),
        "lnb": np.ascontiguousarray(np.broadcast_to(np.asarray(ln_b, np.float32), (128, EMB))),
        "Wc": np.ascontiguousarray(W_cls, dtype=np.float32),
    }
    in_maps = []
    for core in range(N_CORES):
        b, ib = core // 4, core % 4
        S_core = np.concatenate(
            [S_b[b][:, ib * IB:(ib + 1) * IB], S_b[b], cls_col], axis=1)
        in_maps.append({
            "seq": np.ascontiguousarray(seq[b].astype(np.float16)),
            "S": np.ascontiguousarray(S_core.astype(np.float16)),
            **shared,
        })
    return in_maps


def kernel(**inputs) -> np.ndarray:
    nc = _get_module()
    in_maps = _build_inputs(**inputs)
    res = run_bass_kernel_spmd(nc, in_maps, core_ids=list(range(N_CORES)))
    outs = np.stack([res.results[c]["out"] for c in range(N_CORES)])  # [8,256,97]
    return outs.reshape(B, 4, IB, E, NL).reshape(B, E, E, NL)



# revision 37
# speedup vs baseline: 1.5325x; 1.0290x over previous
"""DocRE model kernel for 8 Trainium2 NeuronCores.

Data-parallel over the pair grid: core = b*4 + ib owns document b and
i-rows [8*ib, 8*ib+8) of the 32x32 entity-pair grid (256 pairs/core).
All weights are replicated; W_ext (49152x768) is streamed from HBM
through a float32r matmul with the group-bilinear feature tiles
materialized on-chip.
"""

import numpy as np

import concourse.bacc as bacc
import concourse.bass as bass
import concourse.tile as tile
from concourse import mybir
from concourse.bass_utils import run_bass_kernel_spmd
from concourse.masks import make_identity

F32 = mybir.dt.float32
F32R = mybir.dt.float32r
F16 = mybir.dt.float16

B, L, H = 2, 1024, 768
E, M = 32, 4
EMB, BLK, NL = 768, 64, 97
G = EMB // BLK  # 12
LN_EPS = 1e-12

N_CORES = 8
IB = E // (N_CORES // B)     # 8 i-rows per core
NPAIR = IB * E               # 256 pairs per core
PT = NPAIR // 128            # 2 pair-tiles
KT = EMB * BLK // 128        # 384 k-tiles
CT = EMB // 128              # 6 feature chunks
KC = H // 128                # 6 contraction chunks of H
LC = L // 128                # 8 chunks of L
NENT = IB + E + 1            # 41 cols: [my 8 entities | all 32 | cls]


def _build_module():
    nc = bacc.Bacc("TRN2", target_bir_lowering=False, debug=False)

    seq_d = nc.dram_tensor("seq", [L, H], F16, kind="ExternalInput")
    S_d = nc.dram_tensor("S", [L, NENT], F16, kind="ExternalInput")
    Wh_d = nc.dram_tensor("Wh", [3 * H, EMB], F16, kind="ExternalInput")
    Wt_d = nc.dram_tensor("Wt", [3 * H, EMB], F16, kind="ExternalInput")
    bh_d = nc.dram_tensor("bh", [128, CT], F32, kind="ExternalInput")
    bt_d = nc.dram_tensor("bt", [128, CT], F32, kind="ExternalInput")
    Wx_d = nc.dram_tensor("Wx", [EMB * BLK, EMB], F16, kind="ExternalInput")
    Ebc_d = nc.dram_tensor("Ebc", [2, 128], F16, kind="ExternalInput")
    bx_d = nc.dram_tensor("bx", [128, EMB], F32, kind="ExternalInput")
    Sc_d = nc.dram_tensor("Sc", [128, 2, NL], F32, kind="ExternalInput")
    Wc_d = nc.dram_tensor("Wc", [EMB, NL], F32, kind="ExternalInput")
    out_d = nc.dram_tensor("out", [NPAIR, NL], F32, kind="ExternalOutput")

    with tile.TileContext(nc) as tc:
        with (
            tc.tile_pool(name="persist", bufs=1) as persist,
            tc.tile_pool(name="seqp", bufs=1) as seqp,
            tc.tile_pool(name="whp", bufs=6) as whp,
            tc.tile_pool(name="wxp", bufs=10) as wxp,
            tc.tile_pool(name="blp", bufs=8) as blp,
            tc.tile_pool(name="hsgp", bufs=4) as hsgp,
            tc.tile_pool(name="tmpp", bufs=3) as tmpp,
            tc.tile_pool(name="dramp", bufs=1, space="DRAM") as dramp,
            tc.tile_pool(name="psf", bufs=1, space="PSUM") as psf,
            tc.tile_pool(name="psg", bufs=2, space="PSUM") as psg,
            tc.tile_pool(name="psb", bufs=2, space="PSUM") as psb,
        ):
            ident = persist.tile([128, 128], F32, name="ident")
            make_identity(nc, ident[:])

            # ---- per-column constants broadcast to all partitions ----
            # (allocated here; the DMAs are issued after phase P so they
            # don't delay the phase E/A input streams at the head)
            bx_b = persist.tile([128, EMB], F32, name="bx_b")
            sc_b = persist.tile([128, 2, NL], F32, name="sc_b")

            eps_t = persist.tile([128, 1], F32, name="eps")
            nc.vector.memset(eps_t[:], LN_EPS)

            # ---- phase E: entity pooling  ent = S^T @ seq ----
            # inputs first: S in one DMA, seq in two, so the first matmul's
            # inputs aren't stuck behind a queue of constant loads.
            seq_t = seqp.tile([128, LC, H], F16, name="seq_t")
            S_t = seqp.tile([128, LC, NENT], F16, name="S_t")
            seq_re = seq_d.ap().rearrange("(c p) h -> p c h", p=128)
            S_re = S_d.ap().rearrange("(c p) n -> p c n", p=128)
            nc.sync.dma_start(S_t[:], S_re)
            nc.sync.dma_start(seq_t[:, 0:LC // 2, :], seq_re[:, 0:LC // 2, :])
            nc.sync.dma_start(seq_t[:, LC // 2:LC, :], seq_re[:, LC // 2:LC, :])

            E_t = persist.tile([2, 128], F16, name="E_t")
            nc.sync.dma_start(E_t[:], Ebc_d.ap())
            # per-partition bias chunks bh/bt: [128, CT]
            bh_t = persist.tile([128, CT], F32, name="bh_t")
            bt_t = persist.tile([128, CT], F32, name="bt_t")
            for tile_, src in ((bh_t, bh_d), (bt_t, bt_d)):
                nc.sync.dma_start(tile_[:], src.ap())

            ps_e0 = psg.tile([NENT, 512], F32, name="gen")
            ps_e1 = psg.tile([NENT, 256], F32, name="gen")
            for kc in range(LC):
                nc.tensor.matmul(ps_e0[:], S_t[:, kc, :], seq_t[:, kc, 0:512],
                                 start=(kc == 0), stop=(kc == LC - 1))
                nc.tensor.matmul(ps_e1[:], S_t[:, kc, :], seq_t[:, kc, 512:768],
                                 start=(kc == 0), stop=(kc == LC - 1))
            ent_nat = persist.tile([NENT, H], F32, name="ent_nat")
            nc.scalar.copy(ent_nat[:, 0:512], ps_e0[:])
            nc.scalar.copy(ent_nat[:, 512:768], ps_e1[:])

            # transpose ent -> entT [h, NENT]  (f32r: feeds phase-A matmuls)
            entT = persist.tile([128, KC, NENT], F16, name="entT")
            for kc in range(KC):
                ps_tr = psg.tile([128, NENT], F32, name="gen")
                nc.tensor.transpose(ps_tr[:], ent_nat[:, kc * 128:(kc + 1) * 128],
                                    ident[:NENT, :NENT])
                nc.scalar.copy(entT[:, kc, :], ps_tr[:])

            # ---- phase A: A/B/C projections, software-pipelined ----
            # natural layout first: X_nat = ent @ W_block  [41, 768], then
            # PE-transpose into ABCD[ct][:, m, :] ([c,41], m: Ah,Bh,At,Bt).
            # Four PSUM accumulator pairs (psg + both ps_feat + psb) keep up
            # to four chains in flight; each chain's post-processing is
            # emitted after the NEXT chain's matmuls so the PE stays busy
            # (and HAM-warm) while the chain weight DMAs stream in.
            ABCD = []
            for ct in range(CT):
                abcd_alloc = persist.tile([128, 4, NENT + 1], F32, name=f"abcd{ct}")
                nc.vector.memset(abcd_alloc[:], 0.0)
                ABCD.append(abcd_alloc)

            ps_feat = [[psf.tile([128, 512], F32, name=f"pf{pt}a"),
                        psf.tile([128, 256], F32, name=f"pf{pt}b")]
                       for pt in range(PT)]

            hsT = persist.tile([128, CT, 2 * 128], F16, name="hsT")
            tsdup = persist.tile([128, G, 2 * 128], F16, name="tsdup")
            hs_dram = dramp.tile([EMB, 2 * 128], F16, name="hs_dram")
            NE2 = NENT + 1

            def colview(tile_, m, col0, ap_pat):
                return bass.AP(tensor=tile_.tensor,
                               offset=tile_.offset + m * NE2 + col0,
                               ap=[tile_.ap[0]] + ap_pat)

            def chain_mms(w_d, blk, pair, cls_only=False):
                ps_n0, ps_n1 = pair
                c0, c1 = (IB + E, IB + E + 1) if cls_only else (0, NENT)
                mrows = 1 if cls_only else NENT
                for kc2 in range(KC // 2):
                    w_t = whp.tile([128, 2, EMB], F16, name="w_t")
                    nc.sync.dma_start(
                        w_t[:],
                        w_d.ap()[blk * H + kc2 * 256: blk * H + (kc2 + 1) * 256, :]
                        .rearrange("(q p) e -> p q e", q=2))
                    for qq in range(2):
                        kc = 2 * kc2 + qq
                        nc.tensor.matmul(ps_n0[:mrows, 0:512],
                                         entT[:, kc, c0:c1], w_t[:, qq, 0:512],
                                         start=(kc == 0), stop=(kc == KC - 1))
                        nc.tensor.matmul(ps_n1[:mrows, 0:256],
                                         entT[:, kc, c0:c1], w_t[:, qq, 512:768],
                                         start=(kc == 0), stop=(kc == KC - 1))

            def ab_post(m, pair):
                ps_n0, ps_n1 = pair
                x_nat = tmpp.tile([NENT, EMB], F32, name="x_nat")
                nc.scalar.copy(x_nat[:, 0:512], ps_n0[:NENT, 0:512])
                nc.scalar.copy(x_nat[:, 512:768], ps_n1[:NENT, 0:256])
                for ct in range(CT):
                    ps_tr = (ps_n0 if ct % 2 == 0 else ps_n1)[:, 0:NENT]
                    nc.tensor.transpose(ps_tr, x_nat[:, ct * 128:(ct + 1) * 128],
                                        ident[:NENT, :NENT])
                    nc.scalar.copy(ABCD[ct][:, m, 0:NENT], ps_tr)

            def c_post(m_sel, pair, bias_t):
                ps_c0, ps_c1 = pair
                c_nat = tmpp.tile([1, EMB], F32, name="c_nat")
                nc.scalar.copy(c_nat[:, 0:512], ps_c0[:1, 0:512])
                nc.scalar.copy(c_nat[:, 512:768], ps_c1[:1, 0:256])
                for ct in range(CT):
                    ps_tr = (ps_c0 if ct % 2 == 0 else ps_c1)[:, 0:1]
                    nc.tensor.transpose(ps_tr,
                                        c_nat[:, ct * 128:(ct + 1) * 128],
                                        ident[:1, :1])
                    nc.vector.tensor_tensor(ABCD[ct][:, m_sel, NENT:NENT + 1],
                                            ps_tr,
                                            bias_t[:, ct:ct + 1],
                                            op=mybir.AluOpType.add)

            def ts_post(ct):
                abcd_t = ABCD[ct]
                for half in range(2):
                    g = 2 * ct + half
                    dup_t = tmpp.tile([128, 4, NE2], F32, name="dup")
                    src_ab = abcd_t[half * 64:half * 64 + 64, :, :]
                    nc.scalar.dma_start(dup_t[0:64, :, :], src_ab)
                    nc.scalar.dma_start(dup_t[64:128, :, :], src_ab)
                    tmp2 = tmpp.tile([128, 8, 32], F32, name="tmp")
                    nc.vector.tensor_tensor(
                        tmp2[:], colview(dup_t, 2, IB, [[0, 8], [1, 32]]),
                        colview(dup_t, 3, 0, [[1, 8], [0, 32]]),
                        op=mybir.AluOpType.add)
                    nc.scalar.activation(
                        tsdup[:, g, :].rearrange("p (a b) -> p a b", a=8),
                        tmp2[:], mybir.ActivationFunctionType.Tanh,
                        bias=dup_t[:, 3, NENT:NENT + 1], scale=1.0)

            def hs_post(ct):
                abcd_t = ABCD[ct]
                tmp = tmpp.tile([128, 8, 32], F32, name="tmp")
                nc.vector.tensor_tensor(
                    tmp[:], colview(abcd_t, 0, 0, [[1, 8], [0, 32]]),
                    colview(abcd_t, 1, IB, [[0, 8], [1, 32]]),
                    op=mybir.AluOpType.add)
                nc.scalar.activation(
                    hsT[:, ct, :].rearrange("p (a b) -> p a b", a=8),
                    tmp[:], mybir.ActivationFunctionType.Tanh,
                    bias=abcd_t[:, 0, NENT:NENT + 1], scale=1.0)
                nc.scalar.dma_start(hs_dram[ct * 128:(ct + 1) * 128, :],
                                    hsT[:, ct, :])

            pairA = (psg.tile([128, 512], F32, name="gen"),
                     psg.tile([128, 256], F32, name="gen"))
            pairD = (psb.tile([128, 512], F32, name="bc_ps"),
                     psb.tile([128, 512], F32, name="bc_ps"))

            chain_mms(Wt_d, 2, pairA, cls_only=True)    # c_t
            chain_mms(Wt_d, 0, ps_feat[0])              # At
            c_post(3, pairA, bt_t)
            chain_mms(Wt_d, 1, ps_feat[1])              # Bt
            ab_post(2, ps_feat[0])
            chain_mms(Wh_d, 2, pairD, cls_only=True)    # c_h
            ab_post(3, ps_feat[1])
            for ct in range(CT):
                ts_post(ct)
            chain_mms(Wh_d, 0, ps_feat[0])              # Ah
            c_post(0, pairD, bh_t)
            chain_mms(Wh_d, 1, ps_feat[1])              # Bh
            ab_post(0, ps_feat[0])
            ab_post(1, ps_feat[1])

            hsg_tiles = {}

            def stage_group(g):
                # stage group g's 64 hs rows into partitions 0-1, pair-major:
                # hsg[r, tq, 256*q + p] = hs row (g*64 + 4*tq + 2*q + r)
                hsg_tiles[g] = hsgp.tile([2, 16, 512], F16, name="hsg")
                nc.scalar.dma_start(
                    hsg_tiles[g][:].rearrange("r tq (q p) -> r tq q p", q=2),
                    bass.AP(tensor=hs_dram.tensor,
                            offset=hs_dram.offset + g * 64 * 2 * 128,
                            ap=[[256, 2], [4 * 256, 16], [2 * 256, 2], [1, 256]]))

            # stage the first two groups as soon as their hs rows exist so
            # phase M isn't stuck behind the remaining hs chunks.
            hs_post(0)
            stage_group(0)
            stage_group(1)
            for ct in range(1, CT):
                hs_post(ct)

            # phase-L constants: issued now so they queue behind the
            # phase E/A input streams but ahead of the bulk of W_ext.
            for tile_, src in ((bx_b, bx_d), (sc_b, Sc_d)):
                nc.sync.dma_start(tile_[:], src.ap())

            # ---- phase M: main contraction over W_ext ----
            # software pipeline: broadcasts for upcoming k-tiles issue before
            # the current k-tiles' main matmuls so the DVE multiply latency is
            # hidden behind PE work.  Broadcast matmuls are emitted two at a
            # time so they share one E_t LDWEIGHTS; main matmuls stream
            # 256-col chunks (measured faster per column than 512-col).

            def emit_bc_quad(ktq):
                # two [2,512] rhs matmuls sharing one E_t weight load;
                # covers k-tiles 4*ktq .. 4*ktq+3
                kt0 = 4 * ktq
                g = kt0 // 32
                tq = (kt0 % 32) // 2
                if kt0 % 32 == 24 and g + 1 < G and g + 1 not in hsg_tiles:
                    stage_group(g + 1)
                bc_a = psb.tile([128, 512], F32, name="bc_ps")
                bc_b = psb.tile([128, 512], F32, name="bc_ps")
                nc.tensor.matmul(bc_a[:], E_t[:], hsg_tiles[g][:, tq, :],
                                 start=True, stop=True)
                nc.tensor.matmul(bc_b[:], E_t[:], hsg_tiles[g][:, tq + 1, :],
                                 start=True, stop=True)
                return bc_a, bc_b

            bc_cur = emit_bc_quad(0)
            for kt2 in range(KT // 2):
                wx_t = wxp.tile([128, 2, EMB], F16, name="wx_t")
                nc.sync.dma_start(
                    wx_t[:],
                    Wx_d.ap()[kt2 * 256:(kt2 + 1) * 256, :]
                    .rearrange("(q p) e -> p q e", q=2))
                for q in range(2):
                    kt = 2 * kt2 + q
                    g = kt // 32
                    bc_ps = bc_cur[kt2 % 2]
                    bl_t = blp.tile([128, 2 * 128], F16, name="bl_t")
                    nc.vector.tensor_tensor(bl_t[:],
                                            bc_ps[:, q * 256:(q + 1) * 256],
                                            tsdup[:, g, :],
                                            op=mybir.AluOpType.mult)
                    if kt % 4 == 3 and kt + 1 < KT:
                        bc_cur = emit_bc_quad((kt + 1) // 4)
                    first, last = (kt == 0), (kt == KT - 1)
                    for pt in range(PT):
                        lhsT = bl_t[:, pt * 128:(pt + 1) * 128]
                        if first or last:
                            nc.tensor.matmul(ps_feat[pt][0][:], lhsT,
                                             wx_t[:, q, 0:512],
                                             start=first, stop=last)
                        else:
                            nc.tensor.matmul(ps_feat[pt][0][:, 0:256], lhsT,
                                             wx_t[:, q, 0:256],
                                             start=False, stop=False)
                            nc.tensor.matmul(ps_feat[pt][0][:, 256:512], lhsT,
                                             wx_t[:, q, 256:512],
                                             start=False, stop=False)
                        nc.tensor.matmul(ps_feat[pt][1][:], lhsT,
                                         wx_t[:, q, 512:768],
                                         start=first, stop=last)

            # ---- phase L: bias, relu, stats, classifier (LN folded) ----
            # logits = rstd*(feat^T @ (lng.Wc)) - (mean*rstd)*S1 + const
            # with S1 = sum_e lng[e]*Wc[e,:] and const = lnb @ Wc computed
            # on the host, so the PE transpose+classifier path runs in
            # parallel with the DVE mean/var path.
            wc_t = persist.tile([128, CT, NL], F32, name="wc_t")
            nc.sync.dma_start(wc_t[:], Wc_d.ap().rearrange("(c p) n -> p c n", p=128))

            for pt in range(PT):
                feat = persist.tile([128, EMB], F32, name=f"feat{pt}")
                nc.vector.tensor_tensor(feat[:, 0:512], ps_feat[pt][0][:],
                                        bx_b[:, 0:512], op=mybir.AluOpType.add)
                nc.vector.tensor_tensor(feat[:, 512:768], ps_feat[pt][1][:],
                                        bx_b[:, 512:768], op=mybir.AluOpType.add)
                nc.scalar.activation(feat[:], feat[:],
                                     mybir.ActivationFunctionType.Relu,
                                     bias=0.0, scale=1.0)

                # PE path: transpose feat chunks, classifier accumulate
                fT = persist.tile([128, CT, 128], F32, name=f"fT{pt}")
                for ct in range(CT):
                    ps_tr2 = psg.tile([128, 128], F32, name="gen")
                    nc.tensor.transpose(ps_tr2[:], feat[:, ct * 128:(ct + 1) * 128],
                                        ident[:])
                    nc.scalar.copy(fT[:, ct, :], ps_tr2[:])
                ps_lg = psb.tile([128, 512], F32, name="bc_ps")
                for ct in range(CT):
                    nc.tensor.matmul(ps_lg[:, 0:NL], fT[:, ct, :], wc_t[:, ct, :],
                                     start=(ct == 0), stop=(ct == CT - 1))

                # DVE/ACT path: mean & rstd (overlaps the PE path)
                stats = tmpp.tile([128, 3, 6], F32, name="stats")
                f_re = feat.rearrange("p (c f) -> p c f", c=3)
                for c in range(3):
                    nc.vector.bn_stats(stats[:, c, :], f_re[:, c, :])
                mv = tmpp.tile([128, 2], F32, name="mv")
                nc.vector.bn_aggr(mv[:], stats[:])
                sd = tmpp.tile([128, 1], F32, name="sd")
                nc.scalar.activation(sd[:], mv[:, 1:2],
                                     mybir.ActivationFunctionType.Sqrt,
                                     bias=eps_t[:], scale=1.0)
                rstd = tmpp.tile([128, 1], F32, name="rstd")
                nc.vector.reciprocal(rstd[:], sd[:])
                mrs = tmpp.tile([128, 1], F32, name="mrs")
                nc.vector.tensor_tensor(mrs[:], mv[:, 0:1], rstd[:],
                                        op=mybir.AluOpType.mult)

                o1 = tmpp.tile([128, NL], F32, name="o1")
                nc.vector.tensor_scalar(o1[:], ps_lg[:, 0:NL], rstd[:], None,
                                        op0=mybir.AluOpType.mult)
                o2 = tmpp.tile([128, NL], F32, name="o2")
                nc.vector.tensor_scalar(o2[:], sc_b[:, 0, :], mrs[:], None,
                                        op0=mybir.AluOpType.mult)
                nc.vector.tensor_tensor(o1[:], o1[:], o2[:],
                                        op=mybir.AluOpType.subtract)
                out_sb = tmpp.tile([128, NL], F32, name="out_sb")
                nc.vector.tensor_tensor(out_sb[:], o1[:], sc_b[:, 1, :],
                                        op=mybir.AluOpType.add)
                nc.scalar.dma_start(out_d.ap()[pt * 128:(pt + 1) * 128, :], out_sb[:])

    nc.compile()
    return nc


_NC_CACHE = []


def _get_module():
    if not _NC_CACHE:
        _NC_CACHE.append(_build_module())
    return _NC_CACHE[0]


_EBC = np.zeros((2, 128), np.float16)
_EBC[0, :64] = 1.0
_EBC[1, 64:] = 1.0


def _build_inputs(seq, starts, ends, mention_mask, W_head, b_head, W_tail, b_tail,
                  W_ext, b_ext, ln_g, ln_b, W_cls):
    seq = np.asarray(seq, np.float32)
    starts = np.asarray(starts, np.int64)
    ends = np.asarray(ends, np.int64)
    mask = np.asarray(mention_mask, np.float32)

    # per-document entity selection matrix: ent = Sb^T @ seq[b]
    S_b = np.zeros((B, L, E), np.float32)
    denom = np.maximum(mask.sum(axis=2), 1.0)          # [B, E]
    w = mask * 0.5 / denom[:, :, None]                 # [B, E, M]
    for b in range(B):
        for e in range(E):
            np.add.at(S_b[b, :, e], starts[b, e] + 1, w[b, e])
            np.add.at(S_b[b, :, e], ends[b, e], w[b, e])

    cls_col = np.zeros((L, 1), np.float32)
    cls_col[0, 0] = 1.0

    lngWc = np.asarray(ln_g, np.float32)[:, None] * np.asarray(W_cls, np.float32)
    S1 = lngWc.sum(axis=0)                                       # [NL]
    cb = np.asarray(ln_b, np.float32) @ np.asarray(W_cls, np.float32)
    Sc = np.ascontiguousarray(
        np.broadcast_to(np.stack([S1, cb])[None, :, :], (128, 2, NL)),
        dtype=np.float32)

    shared = {
        "Wh": np.ascontiguousarray(np.asarray(W_head, np.float32).astype(np.float16)),
        "Wt": np.ascontiguousarray(np.asarray(W_tail, np.float32).astype(np.float16)),
        "bh": np.ascontiguousarray(np.asarray(b_head, np.float32).reshape(CT, 128).T),
        "bt": np.ascontiguousarray(np.asarray(b_tail, np.float32).reshape(CT, 128).T),
        "Wx": np.ascontiguousarray(np.asarray(W_ext).astype(np.float16)),
        "Ebc": _EBC,
        "bx": np.ascontiguousarray(np.broadcast_to(np.asarray(b_ext, np.float32), (128, EMB))),
        "Sc": Sc,
        "Wc": np.ascontiguousarray(lngWc, dtype=np.float32),
    }
    in_maps = []
    for core in range(N_CORES):
        b, ib = core // 4, core % 4
        S_core = np.concatenate(
            [S_b[b][:, ib * IB:(ib + 1) * IB], S_b[b], cls_col], axis=1)
        in_maps.append({
            "seq": np.ascontiguousarray(seq[b].astype(np.float16)),
            "S": np.ascontiguousarray(S_core.astype(np.float16)),
            **shared,
        })
    return in_maps


def kernel(**inputs) -> np.ndarray:
    nc = _get_module()
    in_maps = _build_inputs(**inputs)
    res = run_bass_kernel_spmd(nc, in_maps, core_ids=list(range(N_CORES)))
    outs = np.stack([res.results[c]["out"] for c in range(N_CORES)])  # [8,256,97]
    return outs.reshape(B, 4, IB, E, NL).reshape(B, E, E, NL)

